# revision 13
# baseline (speedup 1.0000x reference)
"""DynamicGCN (3-layer GCN + temporal gate) on 8 trn2 NeuronCores via Bass.

Distribution: nodes are partitioned contiguously across the 8 cores (12544
rows each, padded); each core owns the edges whose dst lands in its range
(self-loops become explicit self-edges with norm 1/deg). Per layer:

  1. h' = dinv * (x @ W) for own nodes (PE matmul, ACT eviction applies the
     per-node dinv scale), staged to DRAM.
  2. AllGather of h' in 4 node-slice chunks -> a replicated [100352,128] fp16
     table (each 25088-row chunk doubles as an int16-indexable gather table).
  3. Edge messages: dma_gather pulls h'[src] rows (1024 rows/call, 4 SWDGE
     queues); per 128-edge tile a host-precomputed 0/1 indicator [128,32] is
     the stationary operand of a PE matmul that segment-sums edges into
     per-dst-slot rows (4 tiles col-tiled into one PSUM tile). Since h' rows
     already carry dinv[src] and the remaining dinv[dst] factor is applied
     after aggregation, the indicator needs no weights.
  4. Evicted slot rows are dma_scatter_add-ed into a per-quarter accumulator
     (slots are unique within a quarter, so no RMW collisions; quarter
     accumulators are merged at reload time).
  5. Reload: sum the 4 accumulators, relu with dinv[dst] scale on ACT,
     PE-transpose into the next layer's xT, multiply by the temporal gate.

The temporal gate MLP runs once on-device at kernel start.
"""
import sys, os, types

for _p in ("/opt/trn_rl_repo", os.path.dirname(os.path.abspath(__file__))):
    if _p not in sys.path:
        sys.path.insert(0, _p)

import numpy as np

# ---------------------------------------------------------------- shims ----
def _install_shims():
    # NTFF profile hook (missing module in this container; used for trace=True)
    if "antenv.axon_hooks" not in sys.modules:
        try:
            import antenv
            from trn_agent_boot.trn_boot import _ntff_profile_via_ctypes

            mod = types.ModuleType("antenv.axon_hooks")
            _state = {"hook": None}
            mod.set_axon_ntff_profile_hook = lambda h: _state.__setitem__("hook", h)
            mod.get_axon_ntff_profile_hook = lambda: _state["hook"]
            sys.modules["antenv.axon_hooks"] = mod
            antenv.axon_hooks = mod
            if os.path.exists("/opt/axon/libaxon_pjrt.so"):
                mod.set_axon_ntff_profile_hook(
                    _ntff_profile_via_ctypes("/opt/axon/libaxon_pjrt.so")
                )
        except Exception:
            pass

    # walrus in this container rejects >1 sync wait per instruction; split
    # extra waits onto same-engine NoOps (identical semantics).
    import concourse.bass as bass
    import orjson

    if getattr(bass.Bass.to_json_bytes, "_waitsplit", False):
        return

    orig = bass.Bass.to_json_bytes

    def _split(j):
        ctr = 0
        for fn in j.get("functions", []):
            for bb in fn.get("blocks", []):
                out, changed = [], False
                for ins in bb.get("instructions", []):
                    si = ins.get("sync_info")
                    waits = (si or {}).get("on_wait") or []
                    if len(waits) > 1 and ins.get("engine") not in (None, "Unassigned"):
                        for w in waits[:-1]:
                            ctr += 1
                            out.append({
                                "debug": ins.get("debug", 0), "engine": ins["engine"],
                                "ins": [], "outs": [], "name": f"I-wsplit-{ctr}",
                                "opcode": "NoOp",
                                "sync_info": {"on_update": [], "on_wait": [w]},
                            })
                        si["on_wait"] = [waits[-1]]
                        changed = True
                    out.append(ins)
                if changed:
                    bb["instructions"] = out
        return j

    def to_json_bytes(self):
        return orjson.dumps(_split(orjson.loads(orig(self))))

    to_json_bytes._waitsplit = True
    bass.Bass.to_json_bytes = to_json_bytes


_install_shims()

import concourse.bass as bass
import concourse.mybir as mybir
import concourse.tile as tile
from concourse import library_config
from concourse.bass_utils import run_bass_kernel_spmd

f16 = mybir.dt.float16
f32 = mybir.dt.float32
i16 = mybir.dt.int16

# ---------------------------------------------------------- problem dims ---
N_NODES = 100000
N_EDGES = 600000
D = 128
N_LAYERS = 3
NC = 8
N_OWN = 12544                 # padded rows per core (= 98*128)
N_PAD = N_OWN * NC            # 100352
NQ = 4                        # node-slice quarters (AG chunks / gather tables)
QROWS = N_OWN // NQ           # 3136 rows per rank per quarter
TABROWS = QROWS * NC          # 25088 rows per gather table chunk (< int16 max)
CHUNKS = N_OWN // 128         # 98 matmul chunks
SLOTS = 64                    # dst slots per edge-tile
GROUP = 2                     # tiles per PSUM group (2 x 64 slots = 128)
CALL = 1024                   # rows per dma_gather/scatter call
DUMMY = N_OWN                 # dummy scatter row
XROWS = N_OWN + 128           # accumulator rows (incl. dummy row, 128-aligned)


def _wrap_idx(vals):
    """int16 stream -> [128, n/16] tile layout (16-partition wrap, replicated
    for the 8 Q7 cores). vals length must be a multiple of 16."""
    a = np.asarray(vals, np.int16).reshape(-1, 16).T  # [16, n/16]
    return np.tile(a, (8, 1)).copy()


def _prep_graph(edge_index):
    """Partition/sort/pack edges. Returns per-core input arrays."""
    src = np.asarray(edge_index[0], np.int64)
    dst = np.asarray(edge_index[1], np.int64)
    deg = np.bincount(dst, minlength=N_NODES).astype(np.float32) + 1.0
    dinv = (1.0 / np.sqrt(deg)).astype(np.float32)

    # self-edges
    all_nodes = np.arange(N_NODES, dtype=np.int64)
    s_all = np.concatenate([src, all_nodes])
    d_all = np.concatenate([dst, all_nodes])

    core = d_all // N_OWN
    quarter = (s_all % N_OWN) // QROWS
    # gather-table-local row of the source node
    tab_row = (s_all // N_OWN) * QROWS + (s_all % QROWS)  # rank*3136 + i%3136
    dst_loc = d_all % N_OWN

    # sort by (core, quarter, dst) so per-(core,quarter) runs are dst-grouped
    order = np.lexsort((d_all, quarter, core))
    core, quarter, tab_row, dst_loc = (
        core[order], quarter[order], tab_row[order], dst_loc[order])

    # pass 1: tile counts per (core, quarter)
    per_cq_tiles = np.zeros((NC, NQ), np.int64)
    cq_edges = {}
    for c in range(NC):
        mc = core == c
        for q in range(NQ):
            m = mc & (quarter == q)
            tr, dl = tab_row[m], dst_loc[m]
            # fragment boundaries (dst changes)
            if dl.size:
                bnd = np.nonzero(np.diff(dl))[0] + 1
                starts = np.concatenate([[0], bnd])
                ends = np.concatenate([bnd, [dl.size]])
            else:
                starts = ends = np.zeros(0, np.int64)
            tiles = []  # each: list of (start, end, dst)
            cur, ce, cs = [], 0, 0
            for s0, e0 in zip(starts, ends):
                fl = e0 - s0
                assert fl <= 128, "dst in-degree fragment exceeds one tile"
                if cur and (ce + fl > 128 or cs + 1 > SLOTS):
                    tiles.append(cur)
                    cur, ce, cs = [], 0, 0
                cur.append((int(s0), int(e0), int(dl[s0])))
                ce += fl
                cs += 1
            if cur:
                tiles.append(cur)
            per_cq_tiles[c, q] = len(tiles)
            cq_edges[(c, q)] = (tr, tiles)

    t_q = int(per_cq_tiles.max())
    # round tiles-per-quarter to a multiple of 16 (one scatter call covers
    # 16 tiles' 64 slots; one gather call covers 8 tiles' edges)
    t_q = (t_q + 15) // 16 * 16
    calls_pq = t_q * 128 // CALL

    gidx = np.zeros((NC, NQ, t_q * 128), np.int16)
    sidx = np.full((NC, NQ, t_q * SLOTS), DUMMY, np.int16)
    ind = np.zeros((NC, NQ, t_q, 128, SLOTS), np.float16)
    for c in range(NC):
        for q in range(NQ):
            tr, tiles = cq_edges[(c, q)]
            for t, frags in enumerate(tiles):
                e = 0
                for j, (s0, e0, d_) in enumerate(frags):
                    fl = e0 - s0
                    gidx[c, q, t * 128 + e : t * 128 + e + fl] = tr[s0:e0]
                    ind[c, q, t, e : e + fl, j] = 1.0
                    sidx[c, q, t * SLOTS + j] = d_
                    e += fl
                # remaining gidx rows stay 0 (valid row, indicator 0)
    return dinv, deg, gidx, sidx, ind, t_q, calls_pq


def _build(nc_prog, t_q, calls_pq, has_bias):
    """Emit the bass program. Returns nothing (tensors declared inside)."""
    nc = nc_prog
    t_tot = t_q * NQ
    # ---------------- I/O ----------------
    xT_in = nc.dram_tensor("xT_in", [128, N_OWN], f16, kind="ExternalInput")
    w_in = [nc.dram_tensor(f"w{l}", [128, 128], f16, kind="ExternalInput")
            for l in range(N_LAYERS)]
    dinv_in = nc.dram_tensor("dinv_in", [128, CHUNKS], f32, kind="ExternalInput")
    gidx_in = nc.dram_tensor("gidx_in", [128, t_tot * 8], i16, kind="ExternalInput")
    sidx_in = nc.dram_tensor("sidx_in", [128, t_tot * 4], i16, kind="ExternalInput")
    ind_in = nc.dram_tensor("ind_in", [128, t_tot * SLOTS], f16, kind="ExternalInput")
    ident_in = nc.dram_tensor("ident_in", [128, 128], f16, kind="ExternalInput")
    # gate MLP params
    wg1_in = nc.dram_tensor("wg1_in", [128, 1], f32, kind="ExternalInput")
    bg1_in = nc.dram_tensor("bg1_in", [128, 1], f32, kind="ExternalInput")
    wg2_in = nc.dram_tensor("wg2_in", [128, 128], f32, kind="ExternalInput")
    bg2_in = nc.dram_tensor("bg2_in", [128, 1], f32, kind="ExternalInput")
    ts_in = nc.dram_tensor("ts_in", [128, 1], f32, kind="ExternalInput")
    pref_in = None
    if has_bias:
        pref_in = [nc.dram_tensor(f"pref{l}", [XROWS, 128], f16, kind="ExternalInput")
                   for l in range(N_LAYERS)]
    out_ext = nc.dram_tensor("out_ext", [N_OWN, 128], f32, kind="ExternalOutput")

    # ---------------- internal DRAM ----------------
    h_bounce = nc.dram_tensor("h_bounce", [N_OWN, 128], f16)
    h_tab = nc.dram_tensor("h_tab", [NQ * TABROWS, 128], f16, addr_space="Shared")
    xacc = [nc.dram_tensor(f"xacc{q}", [XROWS, 128], f16) for q in range(NQ)]

    with tile.TileContext(nc) as tc:
        with (
            tc.tile_pool(name="const", bufs=1) as cp,
            tc.tile_pool(name="msgp", bufs=12) as msgp,
            tc.tile_pool(name="stgp", bufs=4) as stgp,
            tc.tile_pool(name="hp", bufs=4) as hp,
            tc.tile_pool(name="rp", bufs=4) as rp,
            tc.tile_pool(name="psum_h", bufs=2, space="PSUM") as pp_h,
            tc.tile_pool(name="psum_seg", bufs=4, space="PSUM") as pp_seg,
            tc.tile_pool(name="psum_t", bufs=2, space="PSUM") as pp_t,
        ):
            nc.gpsimd.load_library(library_config.mlp)
            call_reg = nc.gpsimd.to_reg(CALL)

            # persistent SBUF
            xT = cp.tile([128, N_OWN], f16)
            nc.sync.dma_start(out=xT[:], in_=xT_in[:])
            wt = []
            for l in range(N_LAYERS):
                w = cp.tile([128, 128], f16, tag=f"w{l}")
                nc.sync.dma_start(out=w[:], in_=w_in[l][:])
                wt.append(w)
            dinv_sb = cp.tile([128, CHUNKS], f32)
            nc.sync.dma_start(out=dinv_sb[:], in_=dinv_in[:])
            gidx = cp.tile([128, t_tot * 8], i16)
            nc.sync.dma_start(out=gidx[:], in_=gidx_in[:])
            sidx = cp.tile([128, t_tot * 4], i16)
            nc.sync.dma_start(out=sidx[:], in_=sidx_in[:])
            ident = cp.tile([128, 128], f16)
            nc.sync.dma_start(out=ident[:], in_=ident_in[:])
            zeros8 = cp.tile([128, 8, 128], f16)
            nc.vector.memset(zeros8[:], 0.0)

            # ---------------- temporal gate ----------------
            wg1 = cp.tile([128, 1], f32)
            nc.sync.dma_start(out=wg1[:], in_=wg1_in[:])
            bg1 = cp.tile([128, 1], f32)
            nc.sync.dma_start(out=bg1[:], in_=bg1_in[:])
            wg2 = cp.tile([128, 128], f32)
            nc.sync.dma_start(out=wg2[:], in_=wg2_in[:])
            bg2 = cp.tile([128, 1], f32)
            nc.sync.dma_start(out=bg2[:], in_=bg2_in[:])
            tsr = cp.tile([128, 1], f32)
            nc.sync.dma_start(out=tsr[:], in_=ts_in[:])

            tmp1 = cp.tile([128, 1], f32, tag="g1")
            nc.vector.tensor_mul(tmp1[:], wg1[:], tsr[:])
            tanh1 = cp.tile([128, 1], f32, tag="g2")
            nc.scalar.activation(
                tanh1[:], tmp1[:], mybir.ActivationFunctionType.Tanh, bias=bg1[:])
            ps_g = pp_h.tile([128, 1], f32, tag="psh")
            nc.tensor.matmul(ps_g[:], lhsT=wg2[:], rhs=tanh1[:], start=True, stop=True)
            gate_col = cp.tile([128, 1], f32, tag="gcol")
            nc.scalar.activation(
                gate_col[:], ps_g[:], mybir.ActivationFunctionType.Sigmoid,
                bias=bg2[:])
            gate_col16 = cp.tile([128, 1], f16, tag="gcol16")
            nc.scalar.activation(
                gate_col16[:], ps_g[:], mybir.ActivationFunctionType.Sigmoid,
                bias=bg2[:])
            # replicate gate over partitions: transpose to row, K=1 matmul
            ps_gr = pp_t.tile([1, 128], f16, tag="pst")
            nc.tensor.transpose(out=ps_gr[:], in_=gate_col16[:], identity=ident[:])
            gate_row = cp.tile([1, 128], f16, tag="grow")
            nc.vector.tensor_copy(out=gate_row[:], in_=ps_gr[:])
            ones_row = cp.tile([1, 128], f16, tag="ones")
            nc.vector.memset(ones_row[:], 1.0)
            ps_rep = pp_seg.tile([128, 128], f32, tag="pseg")
            nc.tensor.matmul(
                ps_rep[:], lhsT=ones_row[:], rhs=gate_row[:], start=True, stop=True)
            gate_rep = cp.tile([128, 128], f16, tag="grep")
            nc.scalar.activation(
                gate_rep[:], ps_rep[:], mybir.ActivationFunctionType.Copy)

            # ---------------- layers ----------------
            for l in range(N_LAYERS):
                # h' = dinv * (x @ W)  -> h_bounce
                for c4 in range(CHUNKS // 4 + (1 if CHUNKS % 4 else 0)):
                    n4 = min(4, CHUNKS - c4 * 4)
                    h4 = hp.tile([128, 4, 128], f16, tag="h4")
                    for j in range(n4):
                        c = c4 * 4 + j
                        ps_h = pp_h.tile([128, 128], f32, tag="psh")
                        nc.tensor.matmul(
                            ps_h[:], lhsT=xT[:, c * 128:(c + 1) * 128],
                            rhs=wt[l][:], start=True, stop=True)
                        nc.scalar.activation(
                            h4[:, j, :], ps_h[:],
                            mybir.ActivationFunctionType.Copy,
                            scale=dinv_sb[:, c:c + 1])
                    nc.sync.dma_start(
                        out=h_bounce[c4 * 512:c4 * 512 + n4 * 128, :].rearrange(
                            "(c p) f -> p c f", p=128),
                        in_=h4[:, :n4, :])

                # chunked AllGather: quarter q of every rank -> h_tab chunk q
                for q in range(NQ):
                    nc.gpsimd.collective_compute(
                        "AllGather", mybir.AluOpType.bypass,
                        replica_groups=[list(range(NC))],
                        ins=[h_bounce[q * QROWS:(q + 1) * QROWS, :]],
                        outs=[h_tab[q * TABROWS:(q + 1) * TABROWS, :]],
                    )

                # reset accumulators (or bias prefill)
                for q in range(NQ):
                    if has_bias:
                        for r in range(0, XROWS, 1024):
                            n = min(1024, XROWS - r)
                            t = rp.tile([128, 8, 128], f16, tag="pref")
                            nc.sync.dma_start(
                                out=t[:, :n // 128, :],
                                in_=pref_in[l][r:r + n, :].rearrange(
                                    "(c p) f -> p c f", p=128))
                            nc.sync.dma_start(
                                out=xacc[q][r:r + n, :].rearrange(
                                    "(c p) f -> p c f", p=128),
                                in_=t[:, :n // 128, :])
                    else:
                        for r in range(0, XROWS, 1024):
                            n = min(1024, XROWS - r)
                            nc.sync.dma_start(
                                out=xacc[q][r:r + n, :].rearrange(
                                    "(c p) f -> p c f", p=128),
                                in_=zeros8[:, :n // 128, :])

                # gather -> segment matmul -> scatter, per quarter.
                # One scatter call covers 16 tiles (64 slots each) = 8 PSUM
                # groups = 2 gather calls. stg slice j holds PSUM group j's
                # 128 slot rows (scatter stream position i -> [i%128, i//128]).
                for q in range(NQ):
                    for sc in range(t_q // 16):
                        stg = stgp.tile([128, 8, 128], f16, tag="stg")
                        unit0 = q * t_q + sc * 16
                        indb = msgp.tile([128, 16, SLOTS], f16, tag="indb")
                        nc.sync.dma_start(
                            out=indb[:],
                            in_=ind_in[:, unit0 * SLOTS:(unit0 + 16) * SLOTS]
                            .rearrange("p (t s) -> p t s", s=SLOTS))
                        for gc in range(2):
                            call = sc * 2 + gc
                            tile0 = unit0 + gc * 8
                            msg = msgp.tile([128, 8, 128], f16, tag="msg")
                            gcol0 = tile0 * 8  # int16 cols (128 idx = 8 cols)
                            nc.gpsimd.dma_gather(
                                out_ap=msg[:],
                                in_ap=h_tab[q * TABROWS:(q + 1) * TABROWS, :],
                                idxs_ap=gidx[:, gcol0:gcol0 + 64],
                                num_idxs=CALL, num_idxs_reg=call_reg, elem_size=128,
                                queue_num=call % NQ)
                            for g in range(4):  # 4 psum groups of 2 tiles
                                ps = pp_seg.tile([128, 128], f32, tag="pseg")
                                for j in range(GROUP):
                                    tl = gc * 8 + g * 2 + j
                                    nc.tensor.matmul(
                                        ps[64 * j:64 * (j + 1), :],
                                        lhsT=indb[:, tl, :],
                                        rhs=msg[:, g * 2 + j, :],
                                        start=True, stop=True,
                                        tile_position=(0, 64 * j))
                                nc.scalar.activation(
                                    stg[:, gc * 4 + g, :], ps[:],
                                    mybir.ActivationFunctionType.Copy)
                        scol0 = q * t_q * 4 + sc * 64
                        nc.gpsimd.dma_scatter_add(
                            out_ap=xacc[q][:], in_ap=stg[:],
                            idxs_ap=sidx[:, scol0:scol0 + 64],
                            num_idxs=CALL, num_idxs_reg=call_reg, elem_size=128,
                            queue_num=sc % NQ)

                # reload: merge quarters, relu*dinv, transpose (or final out)
                last = l == N_LAYERS - 1
                for c4 in range(CHUNKS // 4 + (1 if CHUNKS % 4 else 0)):
                    n4 = min(4, CHUNKS - c4 * 4)
                    r0 = c4 * 512
                    nrow = n4 * 128
                    acc = rp.tile([128, 4, 128], f16, tag="acc")
                    for q in range(NQ):
                        t = rp.tile([128, 4, 128], f16, tag=f"ld{q}")
                        nc.sync.dma_start(
                            out=t[:, :n4, :],
                            in_=xacc[q][r0:r0 + nrow, :].rearrange(
                                "(c p) f -> p c f", p=128))
                        if q == 0:
                            nc.vector.tensor_copy(
                                out=acc[:, :n4, :], in_=t[:, :n4, :])
                        else:
                            nc.vector.tensor_add(
                                acc[:, :n4, :], acc[:, :n4, :], t[:, :n4, :])
                    for j in range(n4):
                        c = c4 * 4 + j
                        if last:
                            row32 = rp.tile([128, 128], f32, tag="row32")
                            nc.scalar.activation(
                                row32[:], acc[:, j, :],
                                mybir.ActivationFunctionType.Relu,
                                scale=dinv_sb[:, c:c + 1])
                            rowo = rp.tile([128, 128], f32, tag="rowo")
                            nc.vector.tensor_mul(
                                rowo[:], row32[:], gate_rep[:])
                            nc.sync.dma_start(
                                out=out_ext[c * 128:(c + 1) * 128, :],
                                in_=rowo[:])
                        else:
                            xr2 = rp.tile([128, 128], f16, tag="xr2")
                            nc.scalar.activation(
                                xr2[:], acc[:, j, :],
                                mybir.ActivationFunctionType.Relu,
                                scale=dinv_sb[:, c:c + 1])
                            ps_t = pp_t.tile([128, 128], f16, tag="pst")
                            nc.tensor.transpose(
                                out=ps_t[:], in_=xr2[:], identity=ident[:])
                            nc.vector.tensor_copy(
                                out=xT[:, c * 128:(c + 1) * 128], in_=ps_t[:])
                if not last:
                    # gate: per-feature = per-partition in xT layout
                    nc.vector.tensor_scalar_mul(xT[:], xT[:], gate_col[:])

    mybir.codegen_inst_isa_subclasses(nc)


_CACHE = {}


def _get_program(t_q, calls_pq, has_bias):
    key = (t_q, calls_pq, has_bias)
    if key not in _CACHE:
        nc = bass.Bass(num_devices=NC, num_swdge_queues=NQ)
        _build(nc, t_q, calls_pq, has_bias)
        _CACHE[key] = nc
    return _CACHE[key]


def _prepare(inputs):
    x = np.asarray(inputs["x"], np.float32)
    edge_index = np.asarray(inputs["edge_index"])
    ts = np.asarray(inputs["timestamp"], np.float32).reshape(-1)[0]
    Ws = [np.asarray(inputs[f"W{l}"], np.float32) for l in range(N_LAYERS)]
    bs = [np.asarray(inputs[f"b{l}"], np.float32) for l in range(N_LAYERS)]
    Wg1 = np.asarray(inputs["Wg1"], np.float32)
    bg1 = np.asarray(inputs["bg1"], np.float32)
    Wg2 = np.asarray(inputs["Wg2"], np.float32)
    bg2 = np.asarray(inputs["bg2"], np.float32)

    dinv, deg, gidx, sidx, ind, t_q, calls_pq = _prep_graph(edge_index)
    has_bias = any(np.abs(b).max() > 0 for b in bs)

    ident = np.eye(128, dtype=np.float16)
    in_maps = []
    for c in range(NC):
        lo = c * N_OWN
        hi = min((c + 1) * N_OWN, N_NODES)
        xb = np.zeros((N_OWN, 128), np.float16)
        xb[: hi - lo] = x[lo:hi].astype(np.float16)
        dv = np.ones(N_OWN, np.float32)
        dv[: hi - lo] = dinv[lo:hi]
        m = {
            "xT_in": np.ascontiguousarray(xb.T),
            "dinv_in": np.ascontiguousarray(dv.reshape(CHUNKS, 128).T),
            "gidx_in": np.concatenate(
                [_wrap_idx(gidx[c, q]) for q in range(NQ)], axis=1),
            "sidx_in": np.concatenate(
                [_wrap_idx(sidx[c, q]) for q in range(NQ)], axis=1),
            "ind_in": np.ascontiguousarray(
                ind[c].reshape(NQ * t_q, 128, SLOTS).transpose(1, 0, 2)
                .reshape(128, NQ * t_q * SLOTS)),
            "ident_in": ident,
            "wg1_in": Wg1.reshape(128, 1),
            "bg1_in": bg1.reshape(128, 1),
            "wg2_in": np.ascontiguousarray(Wg2),
            "bg2_in": bg2.reshape(128, 1),
            "ts_in": np.full((128, 1), ts, np.float32),
        }
        for l in range(N_LAYERS):
            m[f"w{l}"] = Ws[l].astype(np.float16)
            if has_bias:
                dvq = np.ones(XROWS, np.float32)
                dvq[:N_OWN] = dv
                pref = (bs[l][None, :] / dvq[:, None]).astype(np.float16)
                pref[N_OWN:] = 0
                m[f"pref{l}"] = pref
        in_maps.append(m)
    return in_maps, t_q, calls_pq, has_bias


def _run(inputs, trace=False):
    in_maps, t_q, calls_pq, has_bias = _prepare(inputs)
    nc = _get_program(t_q, calls_pq, has_bias)
    res = run_bass_kernel_spmd(
        nc, in_maps, core_ids=list(range(NC)), trace=trace)
    blocks = [res.results[c]["out_ext"] for c in range(NC)]
    out = np.concatenate(blocks, axis=0)[:N_NODES].astype(np.float32)
    return out, res


def kernel(**inputs) -> np.ndarray:
    out, _ = _run(inputs, trace=False)
    return out


def kernel_traced(**inputs):
    return _run(inputs, trace=True)


# revision 21
# speedup vs baseline: 2.2278x; 2.2278x over previous
"""DynamicGCN (3-layer GCN + temporal gate) on 8 trn2 NeuronCores via Bass.

Distribution: nodes are partitioned contiguously across the 8 cores (12544
rows each, padded); each core owns the edges whose dst lands in its range
(self-loops become explicit self-edges with norm 1/deg). Per layer:

  1. h' = dinv * (x @ W) for own nodes (PE matmul, ACT eviction applies the
     per-node dinv scale), staged to DRAM.
  2. AllGather of h' in 4 node-slice chunks -> a replicated [100352,128] fp16
     table (each 25088-row chunk doubles as an int16-indexable gather table).
  3. Edge messages: dma_gather pulls h'[src] rows (1024 rows/call, 4 SWDGE
     queues); per 128-edge tile a host-precomputed 0/1 indicator [128,32] is
     the stationary operand of a PE matmul that segment-sums edges into
     per-dst-slot rows (4 tiles col-tiled into one PSUM tile). Since h' rows
     already carry dinv[src] and the remaining dinv[dst] factor is applied
     after aggregation, the indicator needs no weights.
  4. Evicted slot rows are dma_scatter_add-ed into a per-quarter accumulator
     (slots are unique within a quarter, so no RMW collisions; quarter
     accumulators are merged at reload time).
  5. Reload: sum the 4 accumulators, relu with dinv[dst] scale on ACT,
     PE-transpose into the next layer's xT, multiply by the temporal gate.

The temporal gate MLP runs once on-device at kernel start.
"""
import sys, os, types

for _p in ("/opt/trn_rl_repo", os.path.dirname(os.path.abspath(__file__))):
    if _p not in sys.path:
        sys.path.insert(0, _p)

import numpy as np

# ---------------------------------------------------------------- shims ----
def _install_shims():
    # NTFF profile hook (missing module in this container; used for trace=True)
    if "antenv.axon_hooks" not in sys.modules:
        try:
            import antenv
            from trn_agent_boot.trn_boot import _ntff_profile_via_ctypes

            mod = types.ModuleType("antenv.axon_hooks")
            _state = {"hook": None}
            mod.set_axon_ntff_profile_hook = lambda h: _state.__setitem__("hook", h)
            mod.get_axon_ntff_profile_hook = lambda: _state["hook"]
            sys.modules["antenv.axon_hooks"] = mod
            antenv.axon_hooks = mod
            if os.path.exists("/opt/axon/libaxon_pjrt.so"):
                mod.set_axon_ntff_profile_hook(
                    _ntff_profile_via_ctypes("/opt/axon/libaxon_pjrt.so")
                )
        except Exception:
            pass

    # walrus in this container rejects >1 sync wait per instruction; split
    # extra waits onto same-engine NoOps (identical semantics).
    import concourse.bass as bass
    import orjson

    if getattr(bass.Bass.to_json_bytes, "_waitsplit", False):
        return

    orig = bass.Bass.to_json_bytes

    def _split(j):
        ctr = 0
        for fn in j.get("functions", []):
            for bb in fn.get("blocks", []):
                out, changed = [], False
                for ins in bb.get("instructions", []):
                    si = ins.get("sync_info")
                    waits = (si or {}).get("on_wait") or []
                    if len(waits) > 1 and ins.get("engine") not in (None, "Unassigned"):
                        for w in waits[:-1]:
                            ctr += 1
                            out.append({
                                "debug": ins.get("debug", 0), "engine": ins["engine"],
                                "ins": [], "outs": [], "name": f"I-wsplit-{ctr}",
                                "opcode": "NoOp",
                                "sync_info": {"on_update": [], "on_wait": [w]},
                            })
                        si["on_wait"] = [waits[-1]]
                        changed = True
                    out.append(ins)
                if changed:
                    bb["instructions"] = out
        return j

    def to_json_bytes(self):
        return orjson.dumps(_split(orjson.loads(orig(self))))

    to_json_bytes._waitsplit = True
    bass.Bass.to_json_bytes = to_json_bytes


_install_shims()

import concourse.bass as bass
import concourse.mybir as mybir
import concourse.tile as tile
from concourse import library_config
from concourse.bass_utils import run_bass_kernel_spmd

f16 = mybir.dt.float16
f32 = mybir.dt.float32
i16 = mybir.dt.int16

# ---------------------------------------------------------- problem dims ---
N_NODES = 100000
N_EDGES = 600000
D = 128
N_LAYERS = 3
NC = 8
N_OWN = 12544                 # padded rows per core (= 98*128)
N_PAD = N_OWN * NC            # 100352
NQ = 4                        # node-slice quarters (AG chunks / gather tables)
QROWS = N_OWN // NQ           # 3136 rows per rank per quarter
TABROWS = QROWS * NC          # 25088 rows per gather table chunk (< int16 max)
CHUNKS = N_OWN // 128         # 98 matmul chunks
SLOTS = 64                    # dst slots per edge-tile
GROUP = 2                     # tiles per PSUM group (2 x 64 slots = 128)
CALL = 1024                   # rows per dma_gather/scatter call
DUMMY = N_OWN                 # dummy scatter row
XROWS = N_OWN + 128           # accumulator rows (incl. dummy row, 128-aligned)


def _wrap_idx(vals):
    """int16 stream -> [128, n/16] tile layout (16-partition wrap, replicated
    for the 8 Q7 cores). vals length must be a multiple of 16."""
    a = np.asarray(vals, np.int16).reshape(-1, 16).T  # [16, n/16]
    return np.tile(a, (8, 1)).copy()


def _prep_graph(edge_index):
    """Partition/sort/pack edges. Returns per-core input arrays."""
    src = np.asarray(edge_index[0], np.int64)
    dst = np.asarray(edge_index[1], np.int64)
    deg = np.bincount(dst, minlength=N_NODES).astype(np.float32) + 1.0
    dinv = (1.0 / np.sqrt(deg)).astype(np.float32)

    # (self-loop term is folded into the reload phase on-device)
    s_all, d_all = src, dst

    core = d_all // N_OWN
    # node-slice quarters: chunked AllGather k concatenates every rank's
    # rows [k*3136,(k+1)*3136) -> table k row = rank*3136 + i%3136
    quarter = (s_all % N_OWN) // QROWS
    tab_row = (s_all // N_OWN) * QROWS + (s_all % QROWS)
    dst_loc = d_all % N_OWN

    # sort by (core, quarter, dst) so per-(core,quarter) runs are dst-grouped
    order = np.lexsort((d_all, quarter, core))
    core, quarter, tab_row, dst_loc = (
        core[order], quarter[order], tab_row[order], dst_loc[order])

    # pass 1: tile counts per (core, quarter)
    per_cq_tiles = np.zeros((NC, NQ), np.int64)
    cq_edges = {}
    for c in range(NC):
        mc = core == c
        for q in range(NQ):
            m = mc & (quarter == q)
            tr, dl = tab_row[m], dst_loc[m]
            # fragment boundaries (dst changes)
            if dl.size:
                bnd = np.nonzero(np.diff(dl))[0] + 1
                starts = np.concatenate([[0], bnd])
                ends = np.concatenate([bnd, [dl.size]])
            else:
                starts = ends = np.zeros(0, np.int64)
            tiles = []  # each: list of (start, end, dst)
            cur, ce, cs = [], 0, 0
            for s0, e0 in zip(starts, ends):
                fl = e0 - s0
                assert fl <= 128, "dst in-degree fragment exceeds one tile"
                if cur and (ce + fl > 128 or cs + 1 > SLOTS):
                    tiles.append(cur)
                    cur, ce, cs = [], 0, 0
                cur.append((int(s0), int(e0), int(dl[s0])))
                ce += fl
                cs += 1
            if cur:
                tiles.append(cur)
            per_cq_tiles[c, q] = len(tiles)
            cq_edges[(c, q)] = (tr, tiles)

    t_q = int(per_cq_tiles.max())
    # round tiles-per-quarter to a multiple of 16 (one scatter call covers
    # 16 tiles' 64 slots; one gather call covers 8 tiles' edges)
    t_q = (t_q + 15) // 16 * 16
    calls_pq = t_q * 128 // CALL

    zrow = t_q * SLOTS  # zero row in each slot buffer
    gidx = np.zeros((NC, NQ, t_q * 128), np.int16)
    ridx = np.full((NC, NQ, N_OWN), zrow, np.int16)
    ind = np.zeros((NC, NQ, t_q, 128, SLOTS), np.float16)
    for c in range(NC):
        for q in range(NQ):
            tr, tiles = cq_edges[(c, q)]
            for t, frags in enumerate(tiles):
                e = 0
                for j, (s0, e0, d_) in enumerate(frags):
                    fl = e0 - s0
                    gidx[c, q, t * 128 + e : t * 128 + e + fl] = tr[s0:e0]
                    ind[c, q, t, e : e + fl, j] = 1.0
                    ridx[c, q, d_] = t * SLOTS + j
                    e += fl
                # remaining gidx rows stay 0 (valid row, indicator 0)
    return dinv, deg, gidx, ridx, ind, t_q, calls_pq


def _build(nc_prog, t_q, calls_pq, has_bias):
    """Emit the bass program. Returns nothing (tensors declared inside)."""
    nc = nc_prog
    t_tot = t_q * NQ
    # ---------------- I/O ----------------
    xT_in = nc.dram_tensor("xT_in", [128, N_OWN], f16, kind="ExternalInput")
    w_in = [nc.dram_tensor(f"w{l}", [128, 128], f16, kind="ExternalInput")
            for l in range(N_LAYERS)]
    dinv_in = nc.dram_tensor("dinv_in", [128, CHUNKS], f32, kind="ExternalInput")
    gidx_in = nc.dram_tensor("gidx_in", [128, t_tot * 8], i16, kind="ExternalInput")
    ridx_in = nc.dram_tensor("ridx_in", [128, NQ * N_OWN // 16], i16, kind="ExternalInput")
    ind_in = nc.dram_tensor("ind_in", [128, t_tot * SLOTS], f16, kind="ExternalInput")
    ident_in = nc.dram_tensor("ident_in", [128, 128], f16, kind="ExternalInput")
    # gate MLP params
    wg1_in = nc.dram_tensor("wg1_in", [128, 1], f32, kind="ExternalInput")
    bg1_in = nc.dram_tensor("bg1_in", [128, 1], f32, kind="ExternalInput")
    wg2_in = nc.dram_tensor("wg2_in", [128, 128], f32, kind="ExternalInput")
    bg2_in = nc.dram_tensor("bg2_in", [128, 1], f32, kind="ExternalInput")
    ts_in = nc.dram_tensor("ts_in", [128, 1], f32, kind="ExternalInput")
    pref_in = None
    if has_bias:
        pref_in = [nc.dram_tensor(f"pref{l}", [XROWS, 128], f16, kind="ExternalInput")
                   for l in range(N_LAYERS)]
    out_ext = nc.dram_tensor("out_ext", [N_OWN, 128], f32, kind="ExternalOutput")

    # ---------------- internal DRAM ----------------
    h_bounce = nc.dram_tensor("h_bounce", [N_OWN, 128], f16)
    slotbuf = [nc.dram_tensor(f"slotbuf{q}", [t_q * SLOTS + 128, 128], f16)
               for q in range(NQ)]
    h_tab = [nc.dram_tensor(f"h_tab{q}", [TABROWS, 128], f16, addr_space="Shared")
             for q in range(NQ)]

    with tile.TileContext(nc) as tc:
        with (
            tc.tile_pool(name="const", bufs=1) as cp,
            tc.tile_pool(name="msgp", bufs=12) as msgp,
            tc.tile_pool(name="stgp", bufs=8) as stgp,
            tc.tile_pool(name="hp", bufs=4) as hp,
            tc.tile_pool(name="rp", bufs=4) as rp,
            tc.tile_pool(name="psum_h", bufs=2, space="PSUM") as pp_h,
            tc.tile_pool(name="psum_seg", bufs=4, space="PSUM") as pp_seg,
            tc.tile_pool(name="psum_t", bufs=2, space="PSUM") as pp_t,
        ):
            nc.gpsimd.load_library(library_config.mlp)
            call_reg = nc.gpsimd.to_reg(CALL)
            tail_reg = nc.gpsimd.to_reg(N_OWN % CALL)

            # persistent SBUF
            xT = cp.tile([128, N_OWN], f16)
            nc.sync.dma_start(out=xT[:], in_=xT_in[:])
            wt = []
            for l in range(N_LAYERS):
                w = cp.tile([128, 128], f16, tag=f"w{l}")
                nc.sync.dma_start(out=w[:], in_=w_in[l][:])
                wt.append(w)
            dinv_sb = cp.tile([128, CHUNKS], f32)
            nc.sync.dma_start(out=dinv_sb[:], in_=dinv_in[:])
            gidx = cp.tile([128, t_tot * 8], i16)
            nc.sync.dma_start(out=gidx[:], in_=gidx_in[:])
            ridx = cp.tile([128, NQ * N_OWN // 16], i16)
            nc.sync.dma_start(out=ridx[:], in_=ridx_in[:])
            ident = cp.tile([128, 128], f16)
            nc.sync.dma_start(out=ident[:], in_=ident_in[:])
            zeros8 = cp.tile([128, 8, 128], f16)
            nc.vector.memset(zeros8[:], 0.0)
            # zero the slot buffers' zero-row block once
            for q in range(NQ):
                nc.sync.dma_start(
                    out=slotbuf[q][t_q * SLOTS:t_q * SLOTS + 128, :].rearrange(
                        "(c p) f -> p c f", p=128),
                    in_=zeros8[:, :1, :])

            # ---------------- temporal gate ----------------
            wg1 = cp.tile([128, 1], f32)
            nc.sync.dma_start(out=wg1[:], in_=wg1_in[:])
            bg1 = cp.tile([128, 1], f32)
            nc.sync.dma_start(out=bg1[:], in_=bg1_in[:])
            wg2 = cp.tile([128, 128], f32)
            nc.sync.dma_start(out=wg2[:], in_=wg2_in[:])
            bg2 = cp.tile([128, 1], f32)
            nc.sync.dma_start(out=bg2[:], in_=bg2_in[:])
            tsr = cp.tile([128, 1], f32)
            nc.sync.dma_start(out=tsr[:], in_=ts_in[:])

            tmp1 = cp.tile([128, 1], f32, tag="g1")
            nc.vector.tensor_mul(tmp1[:], wg1[:], tsr[:])
            tanh1 = cp.tile([128, 1], f32, tag="g2")
            nc.scalar.activation(
                tanh1[:], tmp1[:], mybir.ActivationFunctionType.Tanh, bias=bg1[:])
            ps_g = pp_h.tile([128, 1], f32, tag="psh")
            nc.tensor.matmul(ps_g[:], lhsT=wg2[:], rhs=tanh1[:], start=True, stop=True)
            gate_col = cp.tile([128, 1], f32, tag="gcol")
            nc.scalar.activation(
                gate_col[:], ps_g[:], mybir.ActivationFunctionType.Sigmoid,
                bias=bg2[:])
            gate_col16 = cp.tile([128, 1], f16, tag="gcol16")
            nc.scalar.activation(
                gate_col16[:], ps_g[:], mybir.ActivationFunctionType.Sigmoid,
                bias=bg2[:])
            # replicate gate over partitions: transpose to row, K=1 matmul
            ps_gr = pp_t.tile([1, 128], f16, tag="pst")
            nc.tensor.transpose(out=ps_gr[:], in_=gate_col16[:], identity=ident[:])
            gate_row = cp.tile([1, 128], f16, tag="grow")
            nc.vector.tensor_copy(out=gate_row[:], in_=ps_gr[:])
            ones_row = cp.tile([1, 128], f16, tag="ones")
            nc.vector.memset(ones_row[:], 1.0)
            ps_rep = pp_seg.tile([128, 128], f32, tag="pseg")
            nc.tensor.matmul(
                ps_rep[:], lhsT=ones_row[:], rhs=gate_row[:], start=True, stop=True)
            gate_rep = cp.tile([128, 128], f16, tag="grep")
            nc.scalar.activation(
                gate_rep[:], ps_rep[:], mybir.ActivationFunctionType.Copy)

            # ---------------- layers ----------------
            for l in range(N_LAYERS):
                # h' = dinv * (x @ W)  -> h_bounce
                for c4 in range(CHUNKS // 4 + (1 if CHUNKS % 4 else 0)):
                    n4 = min(4, CHUNKS - c4 * 4)
                    h4 = hp.tile([128, 4, 128], f16, tag="h4")
                    for j in range(n4):
                        c = c4 * 4 + j
                        ps_h = pp_h.tile([128, 128], f32, tag="psh")
                        nc.tensor.matmul(
                            ps_h[:], lhsT=xT[:, c * 128:(c + 1) * 128],
                            rhs=wt[l][:], start=True, stop=True)
                        nc.scalar.activation(
                            h4[:, j, :], ps_h[:],
                            mybir.ActivationFunctionType.Copy,
                            scale=dinv_sb[:, c:c + 1])
                    nc.sync.dma_start(
                        out=h_bounce[c4 * 512:c4 * 512 + n4 * 128, :].rearrange(
                            "(c p) f -> p c f", p=128),
                        in_=h4[:, :n4, :])

                # chunked AllGather: one per quarter so quarter-q gathers
                # can start as soon as table q lands
                for q in range(NQ):
                    nc.gpsimd.collective_compute(
                        "AllGather", mybir.AluOpType.bypass,
                        replica_groups=[list(range(NC))],
                        ins=[h_bounce[q * QROWS:(q + 1) * QROWS, :]],
                        outs=[h_tab[q][:]],
                    )


                # gather -> segment matmul -> dense slot-row eviction, per quarter.
                # One scatter call covers 16 tiles (64 slots each) = 8 PSUM
                # groups = 2 gather calls. stg slice j holds PSUM group j's
                # 128 slot rows (scatter stream position i -> [i%128, i//128]).
                for q in range(NQ):
                    for sc in range(t_q // 16):
                        stg = stgp.tile([128, 8, 128], f16, tag="stg")
                        unit0 = q * t_q + sc * 16
                        indb = msgp.tile([128, 16, SLOTS], f16, tag="indb")
                        nc.sync.dma_start(
                            out=indb[:],
                            in_=ind_in[:, unit0 * SLOTS:(unit0 + 16) * SLOTS]
                            .rearrange("p (t s) -> p t s", s=SLOTS))
                        for gc in range(2):
                            call = sc * 2 + gc
                            tile0 = unit0 + gc * 8
                            msg = msgp.tile([128, 8, 128], f16, tag="msg")
                            gcol0 = tile0 * 8  # int16 cols (128 idx = 8 cols)
                            nc.gpsimd.dma_gather(
                                out_ap=msg[:],
                                in_ap=h_tab[q][:],
                                idxs_ap=gidx[:, gcol0:gcol0 + 64],
                                num_idxs=CALL, num_idxs_reg=call_reg, elem_size=128,
                                queue_num=call % 2)
                            for g in range(4):  # 4 psum groups of 2 tiles
                                ps = pp_seg.tile([128, 128], f32, tag="pseg")
                                for j in range(GROUP):
                                    tl = gc * 8 + g * 2 + j
                                    nc.tensor.matmul(
                                        ps[64 * j:64 * (j + 1), :],
                                        lhsT=indb[:, tl, :],
                                        rhs=msg[:, g * 2 + j, :],
                                        start=True, stop=True,
                                        tile_position=(0, 64 * j))
                                nc.vector.tensor_copy(
                                    out=stg[:, gc * 4 + g, :], in_=ps[:])
                        (nc.sync if sc % 2 == 0 else nc.scalar).dma_start(
                            out=slotbuf[q][sc * CALL:(sc + 1) * CALL, :].rearrange(
                                "(c p) f -> p c f", p=128),
                            in_=stg[:])

                # reload: gather each node's slot rows from the 4 quarter
                # slot buffers, merge (+ self term h'), relu*dinv, transpose
                last = l == N_LAYERS - 1
                for u in range((N_OWN + CALL - 1) // CALL):
                    r0 = u * CALL
                    nrow = min(CALL, N_OWN - r0)
                    n8 = nrow // 128
                    nreg = call_reg if nrow == CALL else tail_reg
                    th = rp.tile([128, 8, 128], f16, tag="ldh")
                    nc.scalar.dma_start(
                        out=th[:, :n8, :],
                        in_=h_bounce[r0:r0 + nrow, :].rearrange(
                            "(c p) f -> p c f", p=128))
                    acc = rp.tile([128, 8, 128], f16, tag="acc")
                    for q in range(NQ):
                        rg = rp.tile([128, 8, 128], f16, tag=f"rg{q}")
                        rcol0 = (q * N_OWN + r0) // 16
                        nc.gpsimd.dma_gather(
                            out_ap=rg[:, :n8, :],
                            in_ap=slotbuf[q][:],
                            idxs_ap=ridx[:, rcol0:rcol0 + nrow // 16],
                            num_idxs=nrow, num_idxs_reg=nreg, elem_size=128,
                            queue_num=q)
                        if q == 0:
                            nc.vector.tensor_add(
                                acc[:, :n8, :], th[:, :n8, :], rg[:, :n8, :])
                        else:
                            nc.vector.tensor_add(
                                acc[:, :n8, :], acc[:, :n8, :], rg[:, :n8, :])
                    for j in range(n8):
                        c = (r0 // 128) + j
                        if last:
                            row32 = rp.tile([128, 128], f32, tag="row32")
                            nc.scalar.activation(
                                row32[:], acc[:, j, :],
                                mybir.ActivationFunctionType.Relu,
                                scale=dinv_sb[:, c:c + 1])
                            rowo = rp.tile([128, 128], f32, tag="rowo")
                            nc.vector.tensor_mul(
                                rowo[:], row32[:], gate_rep[:])
                            nc.sync.dma_start(
                                out=out_ext[c * 128:(c + 1) * 128, :],
                                in_=rowo[:])
                        else:
                            xr2 = rp.tile([128, 128], f16, tag="xr2")
                            nc.scalar.activation(
                                xr2[:], acc[:, j, :],
                                mybir.ActivationFunctionType.Relu,
                                scale=dinv_sb[:, c:c + 1])
                            ps_t = pp_t.tile([128, 128], f16, tag="pst")
                            nc.tensor.transpose(
                                out=ps_t[:], in_=xr2[:], identity=ident[:])
                            nc.vector.tensor_copy(
                                out=xT[:, c * 128:(c + 1) * 128], in_=ps_t[:])
                if not last:
                    # gate: per-feature = per-partition in xT layout
                    nc.vector.tensor_scalar_mul(xT[:], xT[:], gate_col[:])

    mybir.codegen_inst_isa_subclasses(nc)


_CACHE = {}


def _get_program(t_q, calls_pq, has_bias):
    key = (t_q, calls_pq, has_bias)
    if key not in _CACHE:
        nc = bass.Bass(num_devices=NC, num_swdge_queues=NQ)
        _build(nc, t_q, calls_pq, has_bias)
        _CACHE[key] = nc
    return _CACHE[key]


def _prepare(inputs):
    x = np.asarray(inputs["x"], np.float32)
    edge_index = np.asarray(inputs["edge_index"])
    ts = np.asarray(inputs["timestamp"], np.float32).reshape(-1)[0]
    Ws = [np.asarray(inputs[f"W{l}"], np.float32) for l in range(N_LAYERS)]
    bs = [np.asarray(inputs[f"b{l}"], np.float32) for l in range(N_LAYERS)]
    Wg1 = np.asarray(inputs["Wg1"], np.float32)
    bg1 = np.asarray(inputs["bg1"], np.float32)
    Wg2 = np.asarray(inputs["Wg2"], np.float32)
    bg2 = np.asarray(inputs["bg2"], np.float32)

    dinv, deg, gidx, ridx, ind, t_q, calls_pq = _prep_graph(edge_index)
    has_bias = any(np.abs(b).max() > 0 for b in bs)

    ident = np.eye(128, dtype=np.float16)
    in_maps = []
    for c in range(NC):
        lo = c * N_OWN
        hi = min((c + 1) * N_OWN, N_NODES)
        xb = np.zeros((N_OWN, 128), np.float16)
        xb[: hi - lo] = x[lo:hi].astype(np.float16)
        dv = np.ones(N_OWN, np.float32)
        dv[: hi - lo] = dinv[lo:hi]
        m = {
            "xT_in": np.ascontiguousarray(xb.T),
            "dinv_in": np.ascontiguousarray(dv.reshape(CHUNKS, 128).T),
            "gidx_in": np.concatenate(
                [_wrap_idx(gidx[c, q]) for q in range(NQ)], axis=1),
            "ridx_in": np.concatenate(
                [_wrap_idx(ridx[c, q]) for q in range(NQ)], axis=1),
            "ind_in": np.ascontiguousarray(
                ind[c].reshape(NQ * t_q, 128, SLOTS).transpose(1, 0, 2)
                .reshape(128, NQ * t_q * SLOTS)),
            "ident_in": ident,
            "wg1_in": Wg1.reshape(128, 1),
            "bg1_in": bg1.reshape(128, 1),
            "wg2_in": np.ascontiguousarray(Wg2),
            "bg2_in": bg2.reshape(128, 1),
            "ts_in": np.full((128, 1), ts, np.float32),
        }
        for l in range(N_LAYERS):
            m[f"w{l}"] = Ws[l].astype(np.float16)
            if has_bias:
                dvq = np.ones(XROWS, np.float32)
                dvq[:N_OWN] = dv
                pref = (bs[l][None, :] / dvq[:, None]).astype(np.float16)
                pref[N_OWN:] = 0
                m[f"pref{l}"] = pref
        in_maps.append(m)
    return in_maps, t_q, calls_pq, has_bias


def _run(inputs, trace=False):
    in_maps, t_q, calls_pq, has_bias = _prepare(inputs)
    nc = _get_program(t_q, calls_pq, has_bias)
    res = run_bass_kernel_spmd(
        nc, in_maps, core_ids=list(range(NC)), trace=trace)
    blocks = [res.results[c]["out_ext"] for c in range(NC)]
    out = np.concatenate(blocks, axis=0)[:N_NODES].astype(np.float32)
    return out, res


def kernel(**inputs) -> np.ndarray:
    out, _ = _run(inputs, trace=False)
    return out


def kernel_traced(**inputs):
    return _run(inputs, trace=True)


# revision 25
# speedup vs baseline: 2.3468x; 1.0534x over previous
"""DynamicGCN (3-layer GCN + temporal gate) on 8 trn2 NeuronCores via Bass.

Distribution: nodes are partitioned contiguously across the 8 cores (12544
rows each, padded); each core owns the edges whose dst lands in its range
(self-loops become explicit self-edges with norm 1/deg). Per layer:

  1. h' = dinv * (x @ W) for own nodes (PE matmul, ACT eviction applies the
     per-node dinv scale), staged to DRAM.
  2. AllGather of h' in 4 node-slice chunks -> a replicated [100352,128] fp16
     table (each 25088-row chunk doubles as an int16-indexable gather table).
  3. Edge messages: dma_gather pulls h'[src] rows (1024 rows/call, 4 SWDGE
     queues); per 128-edge tile a host-precomputed 0/1 indicator [128,32] is
     the stationary operand of a PE matmul that segment-sums edges into
     per-dst-slot rows (4 tiles col-tiled into one PSUM tile). Since h' rows
     already carry dinv[src] and the remaining dinv[dst] factor is applied
     after aggregation, the indicator needs no weights.
  4. Evicted slot rows are dma_scatter_add-ed into a per-quarter accumulator
     (slots are unique within a quarter, so no RMW collisions; quarter
     accumulators are merged at reload time).
  5. Reload: sum the 4 accumulators, relu with dinv[dst] scale on ACT,
     PE-transpose into the next layer's xT, multiply by the temporal gate.

The temporal gate MLP runs once on-device at kernel start.
"""
import sys, os, types

for _p in ("/opt/trn_rl_repo", os.path.dirname(os.path.abspath(__file__))):
    if _p not in sys.path:
        sys.path.insert(0, _p)

import numpy as np

# ---------------------------------------------------------------- shims ----
def _install_shims():
    # NTFF profile hook (missing module in this container; used for trace=True)
    if "antenv.axon_hooks" not in sys.modules:
        try:
            import antenv
            from trn_agent_boot.trn_boot import _ntff_profile_via_ctypes

            mod = types.ModuleType("antenv.axon_hooks")
            _state = {"hook": None}
            mod.set_axon_ntff_profile_hook = lambda h: _state.__setitem__("hook", h)
            mod.get_axon_ntff_profile_hook = lambda: _state["hook"]
            sys.modules["antenv.axon_hooks"] = mod
            antenv.axon_hooks = mod
            if os.path.exists("/opt/axon/libaxon_pjrt.so"):
                mod.set_axon_ntff_profile_hook(
                    _ntff_profile_via_ctypes("/opt/axon/libaxon_pjrt.so")
                )
        except Exception:
            pass

    # walrus in this container rejects >1 sync wait per instruction; split
    # extra waits onto same-engine NoOps (identical semantics).
    import concourse.bass as bass
    import orjson

    if getattr(bass.Bass.to_json_bytes, "_waitsplit", False):
        return

    orig = bass.Bass.to_json_bytes

    def _split(j):
        ctr = 0
        for fn in j.get("functions", []):
            for bb in fn.get("blocks", []):
                out, changed = [], False
                for ins in bb.get("instructions", []):
                    si = ins.get("sync_info")
                    waits = (si or {}).get("on_wait") or []
                    if len(waits) > 1 and ins.get("engine") not in (None, "Unassigned"):
                        for w in waits[:-1]:
                            ctr += 1
                            out.append({
                                "debug": ins.get("debug", 0), "engine": ins["engine"],
                                "ins": [], "outs": [], "name": f"I-wsplit-{ctr}",
                                "opcode": "NoOp",
                                "sync_info": {"on_update": [], "on_wait": [w]},
                            })
                        si["on_wait"] = [waits[-1]]
                        changed = True
                    out.append(ins)
                if changed:
                    bb["instructions"] = out
        return j

    def to_json_bytes(self):
        return orjson.dumps(_split(orjson.loads(orig(self))))

    to_json_bytes._waitsplit = True
    bass.Bass.to_json_bytes = to_json_bytes


_install_shims()

import concourse.bass as bass
import concourse.mybir as mybir
import concourse.tile as tile
from concourse import library_config
from concourse.bass_utils import run_bass_kernel_spmd

f16 = mybir.dt.float16
f32 = mybir.dt.float32
i16 = mybir.dt.int16

# ---------------------------------------------------------- problem dims ---
N_NODES = 100000
N_EDGES = 600000
D = 128
N_LAYERS = 3
NC = 8
N_OWN = 12544                 # padded rows per core (= 98*128)
N_PAD = N_OWN * NC            # 100352
NQ = 4                        # node-slice quarters (AG chunks / gather tables)
QROWS = N_OWN // NQ           # 3136 rows per rank per quarter
TABROWS = QROWS * NC          # 25088 rows per gather table chunk (< int16 max)
CHUNKS = N_OWN // 128         # 98 matmul chunks
SLOTS = 64                    # dst slots per edge-tile
GROUP = 2                     # tiles per PSUM group (2 x 64 slots = 128)
CALL = 1024                   # rows per dma_gather/scatter call
DUMMY = N_OWN                 # dummy scatter row
XROWS = N_OWN + 128           # accumulator rows (incl. dummy row, 128-aligned)


def _wrap_idx(vals):
    """int16 stream -> [128, n/16] tile layout (16-partition wrap, replicated
    for the 8 Q7 cores). vals length must be a multiple of 16."""
    a = np.asarray(vals, np.int16).reshape(-1, 16).T  # [16, n/16]
    return np.tile(a, (8, 1)).copy()


def _prep_graph(edge_index):
    """Partition/sort/pack edges. Returns per-core input arrays."""
    src = np.asarray(edge_index[0], np.int64)
    dst = np.asarray(edge_index[1], np.int64)
    deg = np.bincount(dst, minlength=N_NODES).astype(np.float32) + 1.0
    dinv = (1.0 / np.sqrt(deg)).astype(np.float32)

    # (self-loop term is folded into the reload phase on-device)
    s_all, d_all = src, dst

    core = d_all // N_OWN
    # two chunked AllGathers (rows [k*6272,(k+1)*6272) of each rank); each
    # 50176-row rank-major chunk output is split into two contiguous
    # 25088-row tables (ranks 0-3 / 4-7). quarter q = 2*chunk + rankhalf.
    s_rank = s_all // N_OWN
    s_i = s_all % N_OWN
    quarter = 2 * (s_i // (2 * QROWS)) + (s_rank // 4)
    tab_row = (s_rank % 4) * (2 * QROWS) + (s_i % (2 * QROWS))
    dst_loc = d_all % N_OWN

    # sort by (core, quarter, dst) so per-(core,quarter) runs are dst-grouped
    order = np.lexsort((d_all, quarter, core))
    core, quarter, tab_row, dst_loc = (
        core[order], quarter[order], tab_row[order], dst_loc[order])

    # pass 1: tile counts per (core, quarter)
    per_cq_tiles = np.zeros((NC, NQ), np.int64)
    cq_edges = {}
    for c in range(NC):
        mc = core == c
        for q in range(NQ):
            m = mc & (quarter == q)
            tr, dl = tab_row[m], dst_loc[m]
            # fragment boundaries (dst changes)
            if dl.size:
                bnd = np.nonzero(np.diff(dl))[0] + 1
                starts = np.concatenate([[0], bnd])
                ends = np.concatenate([bnd, [dl.size]])
            else:
                starts = ends = np.zeros(0, np.int64)
            tiles = []  # each: list of (start, end, dst)
            cur, ce, cs = [], 0, 0
            for s0, e0 in zip(starts, ends):
                fl = e0 - s0
                assert fl <= 128, "dst in-degree fragment exceeds one tile"
                if cur and (ce + fl > 128 or cs + 1 > SLOTS):
                    tiles.append(cur)
                    cur, ce, cs = [], 0, 0
                cur.append((int(s0), int(e0), int(dl[s0])))
                ce += fl
                cs += 1
            if cur:
                tiles.append(cur)
            per_cq_tiles[c, q] = len(tiles)
            cq_edges[(c, q)] = (tr, tiles)

    t_q = int(per_cq_tiles.max())
    # round tiles-per-quarter to a multiple of 16 (one scatter call covers
    # 16 tiles' 64 slots; one gather call covers 8 tiles' edges)
    t_q = (t_q + 15) // 16 * 16
    calls_pq = t_q * 128 // CALL

    zrow = t_q * SLOTS  # zero row in each slot buffer
    gidx = np.zeros((NC, NQ, t_q * 128), np.int16)
    ridx = np.full((NC, NQ, N_OWN), zrow, np.int16)
    ind = np.zeros((NC, NQ, t_q, 128, SLOTS), np.float16)
    for c in range(NC):
        for q in range(NQ):
            tr, tiles = cq_edges[(c, q)]
            for t, frags in enumerate(tiles):
                e = 0
                for j, (s0, e0, d_) in enumerate(frags):
                    fl = e0 - s0
                    gidx[c, q, t * 128 + e : t * 128 + e + fl] = tr[s0:e0]
                    ind[c, q, t, e : e + fl, j] = 1.0
                    tl = t % 16
                    ridx[c, q, d_] = (
                        (t // 16) * 1024
                        + (64 * (tl % 2) + j) * 8
                        + (tl // 8) * 4 + (tl % 8) // 2)
                    e += fl
                # remaining gidx rows stay 0 (valid row, indicator 0)
    return dinv, deg, gidx, ridx, ind, t_q, calls_pq


def _build(nc_prog, t_q, calls_pq, has_bias):
    """Emit the bass program. Returns nothing (tensors declared inside)."""
    nc = nc_prog
    t_tot = t_q * NQ
    # ---------------- I/O ----------------
    xT_in = nc.dram_tensor("xT_in", [128, N_OWN], f16, kind="ExternalInput")
    w_in = [nc.dram_tensor(f"w{l}", [128, 128], f16, kind="ExternalInput")
            for l in range(N_LAYERS)]
    dinv_in = nc.dram_tensor("dinv_in", [128, CHUNKS], f32, kind="ExternalInput")
    gidx_in = nc.dram_tensor("gidx_in", [128, t_tot * 8], i16, kind="ExternalInput")
    ridx_in = nc.dram_tensor("ridx_in", [128, NQ * N_OWN // 16], i16, kind="ExternalInput")
    ind_in = nc.dram_tensor("ind_in", [128, t_tot * SLOTS], f16, kind="ExternalInput")
    ident_in = nc.dram_tensor("ident_in", [128, 128], f16, kind="ExternalInput")
    # gate MLP params
    wg1_in = nc.dram_tensor("wg1_in", [128, 1], f32, kind="ExternalInput")
    bg1_in = nc.dram_tensor("bg1_in", [128, 1], f32, kind="ExternalInput")
    wg2_in = nc.dram_tensor("wg2_in", [128, 128], f32, kind="ExternalInput")
    bg2_in = nc.dram_tensor("bg2_in", [128, 1], f32, kind="ExternalInput")
    ts_in = nc.dram_tensor("ts_in", [128, 1], f32, kind="ExternalInput")
    pref_in = None
    if has_bias:
        pref_in = [nc.dram_tensor(f"pref{l}", [XROWS, 128], f16, kind="ExternalInput")
                   for l in range(N_LAYERS)]
    out_ext = nc.dram_tensor("out_ext", [N_OWN, 128], f32, kind="ExternalOutput")

    # ---------------- internal DRAM ----------------
    h_bounce = nc.dram_tensor("h_bounce", [N_OWN, 128], f16)
    slotbuf = [nc.dram_tensor(f"slotbuf{q}", [t_q * SLOTS + 128, 128], f16)
               for q in range(NQ)]
    h_chunk = [nc.dram_tensor(f"h_chunk{k}", [2 * TABROWS, 128], f16,
                              addr_space="Shared") for k in range(2)]

    with tile.TileContext(nc) as tc:
        with (
            tc.tile_pool(name="const", bufs=1) as cp,
            tc.tile_pool(name="msgp", bufs=12) as msgp,
            tc.tile_pool(name="stgp", bufs=8) as stgp,
            tc.tile_pool(name="hp", bufs=4) as hp,
            tc.tile_pool(name="rp", bufs=4) as rp,
            tc.tile_pool(name="psum_h", bufs=2, space="PSUM") as pp_h,
            tc.tile_pool(name="psum_seg", bufs=4, space="PSUM") as pp_seg,
            tc.tile_pool(name="psum_t", bufs=2, space="PSUM") as pp_t,
        ):
            nc.gpsimd.load_library(library_config.mlp)
            call_reg = nc.gpsimd.to_reg(CALL)
            tail_reg = nc.gpsimd.to_reg(N_OWN % CALL)

            # persistent SBUF
            xT = cp.tile([128, N_OWN], f16)
            nc.sync.dma_start(out=xT[:], in_=xT_in[:])
            wt = []
            for l in range(N_LAYERS):
                w = cp.tile([128, 128], f16, tag=f"w{l}")
                nc.sync.dma_start(out=w[:], in_=w_in[l][:])
                wt.append(w)
            dinv_sb = cp.tile([128, CHUNKS], f32)
            nc.sync.dma_start(out=dinv_sb[:], in_=dinv_in[:])
            gidx = cp.tile([128, t_tot * 8], i16)
            nc.sync.dma_start(out=gidx[:], in_=gidx_in[:])
            ridx = cp.tile([128, NQ * N_OWN // 16], i16)
            nc.sync.dma_start(out=ridx[:], in_=ridx_in[:])
            ident = cp.tile([128, 128], f16)
            nc.sync.dma_start(out=ident[:], in_=ident_in[:])
            zeros8 = cp.tile([128, 8, 128], f16)
            nc.vector.memset(zeros8[:], 0.0)
            # zero the slot buffers' zero-row block once
            for q in range(NQ):
                nc.sync.dma_start(
                    out=slotbuf[q][t_q * SLOTS:t_q * SLOTS + 128, :].rearrange(
                        "(c p) f -> p c f", p=128),
                    in_=zeros8[:, :1, :])

            # ---------------- temporal gate ----------------
            wg1 = cp.tile([128, 1], f32)
            nc.sync.dma_start(out=wg1[:], in_=wg1_in[:])
            bg1 = cp.tile([128, 1], f32)
            nc.sync.dma_start(out=bg1[:], in_=bg1_in[:])
            wg2 = cp.tile([128, 128], f32)
            nc.sync.dma_start(out=wg2[:], in_=wg2_in[:])
            bg2 = cp.tile([128, 1], f32)
            nc.sync.dma_start(out=bg2[:], in_=bg2_in[:])
            tsr = cp.tile([128, 1], f32)
            nc.sync.dma_start(out=tsr[:], in_=ts_in[:])

            tmp1 = cp.tile([128, 1], f32, tag="g1")
            nc.vector.tensor_mul(tmp1[:], wg1[:], tsr[:])
            tanh1 = cp.tile([128, 1], f32, tag="g2")
            nc.scalar.activation(
                tanh1[:], tmp1[:], mybir.ActivationFunctionType.Tanh, bias=bg1[:])
            ps_g = pp_h.tile([128, 1], f32, tag="psh")
            nc.tensor.matmul(ps_g[:], lhsT=wg2[:], rhs=tanh1[:], start=True, stop=True)
            gate_col = cp.tile([128, 1], f32, tag="gcol")
            nc.scalar.activation(
                gate_col[:], ps_g[:], mybir.ActivationFunctionType.Sigmoid,
                bias=bg2[:])
            gate_col16 = cp.tile([128, 1], f16, tag="gcol16")
            nc.scalar.activation(
                gate_col16[:], ps_g[:], mybir.ActivationFunctionType.Sigmoid,
                bias=bg2[:])
            # replicate gate over partitions: transpose to row, K=1 matmul
            ps_gr = pp_t.tile([1, 128], f16, tag="pst")
            nc.tensor.transpose(out=ps_gr[:], in_=gate_col16[:], identity=ident[:])
            gate_row = cp.tile([1, 128], f16, tag="grow")
            nc.vector.tensor_copy(out=gate_row[:], in_=ps_gr[:])
            ones_row = cp.tile([1, 128], f16, tag="ones")
            nc.vector.memset(ones_row[:], 1.0)
            ps_rep = pp_seg.tile([128, 128], f32, tag="pseg")
            nc.tensor.matmul(
                ps_rep[:], lhsT=ones_row[:], rhs=gate_row[:], start=True, stop=True)
            gate_rep = cp.tile([128, 128], f16, tag="grep")
            nc.scalar.activation(
                gate_rep[:], ps_rep[:], mybir.ActivationFunctionType.Copy)

            # ---------------- layers ----------------
            for l in range(N_LAYERS):
                # h' = dinv * (x @ W)  -> h_bounce
                for c4 in range(CHUNKS // 4 + (1 if CHUNKS % 4 else 0)):
                    n4 = min(4, CHUNKS - c4 * 4)
                    h4 = hp.tile([128, 4, 128], f16, tag="h4")
                    for j in range(n4):
                        c = c4 * 4 + j
                        ps_h = pp_h.tile([128, 128], f32, tag="psh")
                        nc.tensor.matmul(
                            ps_h[:], lhsT=xT[:, c * 128:(c + 1) * 128],
                            rhs=wt[l][:], start=True, stop=True)
                        nc.scalar.activation(
                            h4[:, j, :], ps_h[:],
                            mybir.ActivationFunctionType.Copy,
                            scale=dinv_sb[:, c:c + 1])
                    nc.sync.dma_start(
                        out=h_bounce[c4 * 512:c4 * 512 + n4 * 128, :].rearrange(
                            "(c p) f -> p c f", p=128),
                        in_=h4[:, :n4, :])

                # two chunked AllGathers; gathers for a chunk's two tables
                # start as soon as that chunk lands
                for k in range(2):
                    nc.gpsimd.collective_compute(
                        "AllGather", mybir.AluOpType.bypass,
                        replica_groups=[list(range(NC))],
                        ins=[h_bounce[k * 2 * QROWS:(k + 1) * 2 * QROWS, :]],
                        outs=[h_chunk[k][:]],
                    )


                # gather -> segment matmul -> dense slot-row eviction, per quarter.
                # One scatter call covers 16 tiles (64 slots each) = 8 PSUM
                # groups = 2 gather calls. stg slice j holds PSUM group j's
                # 128 slot rows (scatter stream position i -> [i%128, i//128]).
                for q in range(NQ):
                    for sc in range(t_q // 16):
                        stg = stgp.tile([128, 8, 128], f16, tag="stg")
                        unit0 = q * t_q + sc * 16
                        indb = msgp.tile([128, 16, SLOTS], f16, tag="indb")
                        nc.sync.dma_start(
                            out=indb[:],
                            in_=ind_in[:, unit0 * SLOTS:(unit0 + 16) * SLOTS]
                            .rearrange("p (t s) -> p t s", s=SLOTS))
                        for gc in range(2):
                            call = sc * 2 + gc
                            tile0 = unit0 + gc * 8
                            msg = msgp.tile([128, 8, 128], f16, tag="msg")
                            gcol0 = tile0 * 8  # int16 cols (128 idx = 8 cols)
                            nc.gpsimd.dma_gather(
                                out_ap=msg[:],
                                in_ap=h_chunk[q // 2][
                                    (q % 2) * TABROWS:(q % 2 + 1) * TABROWS, :],
                                idxs_ap=gidx[:, gcol0:gcol0 + 64],
                                num_idxs=CALL, num_idxs_reg=call_reg, elem_size=128,
                                queue_num=call % NQ)
                            for g in range(4):  # 4 psum groups of 2 tiles
                                ps = pp_seg.tile([128, 128], f32, tag="pseg")
                                for j in range(GROUP):
                                    tl = gc * 8 + g * 2 + j
                                    nc.tensor.matmul(
                                        ps[64 * j:64 * (j + 1), :],
                                        lhsT=indb[:, tl, :],
                                        rhs=msg[:, g * 2 + j, :],
                                        start=True, stop=True,
                                        tile_position=(0, 64 * j))
                                nc.vector.tensor_copy(
                                    out=stg[:, gc * 4 + g, :], in_=ps[:])
                        (nc.sync if sc % 2 == 0 else nc.scalar).dma_start(
                            out=slotbuf[q][sc * CALL:(sc + 1) * CALL, :].rearrange(
                                "(p c) f -> p c f", p=128),
                            in_=stg[:])

                # reload: gather each node's slot rows from the 4 quarter
                # slot buffers, merge (+ self term h'), relu*dinv, transpose
                last = l == N_LAYERS - 1
                for u in range((N_OWN + CALL - 1) // CALL):
                    r0 = u * CALL
                    nrow = min(CALL, N_OWN - r0)
                    n8 = nrow // 128
                    nreg = call_reg if nrow == CALL else tail_reg
                    th = rp.tile([128, 8, 128], f16, tag="ldh")
                    nc.scalar.dma_start(
                        out=th[:, :n8, :],
                        in_=h_bounce[r0:r0 + nrow, :].rearrange(
                            "(c p) f -> p c f", p=128))
                    acc = rp.tile([128, 8, 128], f16, tag="acc")
                    for q in range(NQ):
                        rg = rp.tile([128, 8, 128], f16, tag=f"rg{q}")
                        rcol0 = (q * N_OWN + r0) // 16
                        nc.gpsimd.dma_gather(
                            out_ap=rg[:, :n8, :],
                            in_ap=slotbuf[q][:],
                            idxs_ap=ridx[:, rcol0:rcol0 + nrow // 16],
                            num_idxs=nrow, num_idxs_reg=nreg, elem_size=128,
                            queue_num=(u + q) % NQ)
                        if q == 0:
                            nc.vector.tensor_add(
                                acc[:, :n8, :], th[:, :n8, :], rg[:, :n8, :])
                        else:
                            nc.vector.tensor_add(
                                acc[:, :n8, :], acc[:, :n8, :], rg[:, :n8, :])
                    for j in range(n8):
                        c = (r0 // 128) + j
                        if last:
                            row32 = rp.tile([128, 128], f32, tag="row32")
                            nc.scalar.activation(
                                row32[:], acc[:, j, :],
                                mybir.ActivationFunctionType.Relu,
                                scale=dinv_sb[:, c:c + 1])
                            rowo = rp.tile([128, 128], f32, tag="rowo")
                            nc.vector.tensor_mul(
                                rowo[:], row32[:], gate_rep[:])
                            nc.sync.dma_start(
                                out=out_ext[c * 128:(c + 1) * 128, :],
                                in_=rowo[:])
                        else:
                            xr2 = rp.tile([128, 128], f16, tag="xr2")
                            nc.scalar.activation(
                                xr2[:], acc[:, j, :],
                                mybir.ActivationFunctionType.Relu,
                                scale=dinv_sb[:, c:c + 1])
                            ps_t = pp_t.tile([128, 128], f16, tag="pst")
                            nc.tensor.transpose(
                                out=ps_t[:], in_=xr2[:], identity=ident[:])
                            nc.vector.tensor_scalar_mul(
                                xT[:, c * 128:(c + 1) * 128], ps_t[:],
                                gate_col[:])

    mybir.codegen_inst_isa_subclasses(nc)


_CACHE = {}


def _get_program(t_q, calls_pq, has_bias):
    key = (t_q, calls_pq, has_bias)
    if key not in _CACHE:
        nc = bass.Bass(num_devices=NC, num_swdge_queues=NQ)
        _build(nc, t_q, calls_pq, has_bias)
        _CACHE[key] = nc
    return _CACHE[key]


def _prepare(inputs):
    x = np.asarray(inputs["x"], np.float32)
    edge_index = np.asarray(inputs["edge_index"])
    ts = np.asarray(inputs["timestamp"], np.float32).reshape(-1)[0]
    Ws = [np.asarray(inputs[f"W{l}"], np.float32) for l in range(N_LAYERS)]
    bs = [np.asarray(inputs[f"b{l}"], np.float32) for l in range(N_LAYERS)]
    Wg1 = np.asarray(inputs["Wg1"], np.float32)
    bg1 = np.asarray(inputs["bg1"], np.float32)
    Wg2 = np.asarray(inputs["Wg2"], np.float32)
    bg2 = np.asarray(inputs["bg2"], np.float32)

    dinv, deg, gidx, ridx, ind, t_q, calls_pq = _prep_graph(edge_index)
    has_bias = any(np.abs(b).max() > 0 for b in bs)

    ident = np.eye(128, dtype=np.float16)
    in_maps = []
    for c in range(NC):
        lo = c * N_OWN
        hi = min((c + 1) * N_OWN, N_NODES)
        xb = np.zeros((N_OWN, 128), np.float16)
        xb[: hi - lo] = x[lo:hi].astype(np.float16)
        dv = np.ones(N_OWN, np.float32)
        dv[: hi - lo] = dinv[lo:hi]
        m = {
            "xT_in": np.ascontiguousarray(xb.T),
            "dinv_in": np.ascontiguousarray(dv.reshape(CHUNKS, 128).T),
            "gidx_in": np.concatenate(
                [_wrap_idx(gidx[c, q]) for q in range(NQ)], axis=1),
            "ridx_in": np.concatenate(
                [_wrap_idx(ridx[c, q]) for q in range(NQ)], axis=1),
            "ind_in": np.ascontiguousarray(
                ind[c].reshape(NQ * t_q, 128, SLOTS).transpose(1, 0, 2)
                .reshape(128, NQ * t_q * SLOTS)),
            "ident_in": ident,
            "wg1_in": Wg1.reshape(128, 1),
            "bg1_in": bg1.reshape(128, 1),
            "wg2_in": np.ascontiguousarray(Wg2),
            "bg2_in": bg2.reshape(128, 1),
            "ts_in": np.full((128, 1), ts, np.float32),
        }
        for l in range(N_LAYERS):
            m[f"w{l}"] = Ws[l].astype(np.float16)
            if has_bias:
                dvq = np.ones(XROWS, np.float32)
                dvq[:N_OWN] = dv
                pref = (bs[l][None, :] / dvq[:, None]).astype(np.float16)
                pref[N_OWN:] = 0
                m[f"pref{l}"] = pref
        in_maps.append(m)
    return in_maps, t_q, calls_pq, has_bias


def _run(inputs, trace=False):
    in_maps, t_q, calls_pq, has_bias = _prepare(inputs)
    nc = _get_program(t_q, calls_pq, has_bias)
    res = run_bass_kernel_spmd(
        nc, in_maps, core_ids=list(range(NC)), trace=trace)
    blocks = [res.results[c]["out_ext"] for c in range(NC)]
    out = np.concatenate(blocks, axis=0)[:N_NODES].astype(np.float32)
    return out, res


def kernel(**inputs) -> np.ndarray:
    out, _ = _run(inputs, trace=False)
    return out


def kernel_traced(**inputs):
    return _run(inputs, trace=True)


# revision 28
# speedup vs baseline: 2.3749x; 1.0120x over previous
"""DynamicGCN (3-layer GCN + temporal gate) on 8 trn2 NeuronCores via Bass.

Distribution: nodes are partitioned contiguously across the 8 cores (12544
rows each, padded); each core owns the edges whose dst lands in its range
(self-loops become explicit self-edges with norm 1/deg). Per layer:

  1. h' = dinv * (x @ W) for own nodes (PE matmul, ACT eviction applies the
     per-node dinv scale), staged to DRAM.
  2. AllGather of h' in 4 node-slice chunks -> a replicated [100352,128] fp16
     table (each 25088-row chunk doubles as an int16-indexable gather table).
  3. Edge messages: dma_gather pulls h'[src] rows (1024 rows/call, 4 SWDGE
     queues); per 128-edge tile a host-precomputed 0/1 indicator [128,32] is
     the stationary operand of a PE matmul that segment-sums edges into
     per-dst-slot rows (4 tiles col-tiled into one PSUM tile). Since h' rows
     already carry dinv[src] and the remaining dinv[dst] factor is applied
     after aggregation, the indicator needs no weights.
  4. Evicted slot rows are dma_scatter_add-ed into a per-quarter accumulator
     (slots are unique within a quarter, so no RMW collisions; quarter
     accumulators are merged at reload time).
  5. Reload: sum the 4 accumulators, relu with dinv[dst] scale on ACT,
     PE-transpose into the next layer's xT, multiply by the temporal gate.

The temporal gate MLP runs once on-device at kernel start.
"""
import sys, os, types

for _p in ("/opt/trn_rl_repo", os.path.dirname(os.path.abspath(__file__))):
    if _p not in sys.path:
        sys.path.insert(0, _p)

import numpy as np

# ---------------------------------------------------------------- shims ----
def _install_shims():
    # NTFF profile hook (missing module in this container; used for trace=True)
    if "antenv.axon_hooks" not in sys.modules:
        try:
            import antenv
            from trn_agent_boot.trn_boot import _ntff_profile_via_ctypes

            mod = types.ModuleType("antenv.axon_hooks")
            _state = {"hook": None}
            mod.set_axon_ntff_profile_hook = lambda h: _state.__setitem__("hook", h)
            mod.get_axon_ntff_profile_hook = lambda: _state["hook"]
            sys.modules["antenv.axon_hooks"] = mod
            antenv.axon_hooks = mod
            if os.path.exists("/opt/axon/libaxon_pjrt.so"):
                mod.set_axon_ntff_profile_hook(
                    _ntff_profile_via_ctypes("/opt/axon/libaxon_pjrt.so")
                )
        except Exception:
            pass

    # walrus in this container rejects >1 sync wait per instruction; split
    # extra waits onto same-engine NoOps (identical semantics).
    import concourse.bass as bass
    import orjson

    if getattr(bass.Bass.to_json_bytes, "_waitsplit", False):
        return

    orig = bass.Bass.to_json_bytes

    def _split(j):
        ctr = 0
        for fn in j.get("functions", []):
            for bb in fn.get("blocks", []):
                out, changed = [], False
                for ins in bb.get("instructions", []):
                    si = ins.get("sync_info")
                    waits = (si or {}).get("on_wait") or []
                    if len(waits) > 1 and ins.get("engine") not in (None, "Unassigned"):
                        for w in waits[:-1]:
                            ctr += 1
                            out.append({
                                "debug": ins.get("debug", 0), "engine": ins["engine"],
                                "ins": [], "outs": [], "name": f"I-wsplit-{ctr}",
                                "opcode": "NoOp",
                                "sync_info": {"on_update": [], "on_wait": [w]},
                            })
                        si["on_wait"] = [waits[-1]]
                        changed = True
                    out.append(ins)
                if changed:
                    bb["instructions"] = out
        return j

    def to_json_bytes(self):
        return orjson.dumps(_split(orjson.loads(orig(self))))

    to_json_bytes._waitsplit = True
    bass.Bass.to_json_bytes = to_json_bytes


_install_shims()

import concourse.bass as bass
import concourse.mybir as mybir
import concourse.tile as tile
from concourse import library_config
from concourse.bass_utils import run_bass_kernel_spmd

f16 = mybir.dt.float16
f32 = mybir.dt.float32
i16 = mybir.dt.int16

# ---------------------------------------------------------- problem dims ---
N_NODES = 100000
N_EDGES = 600000
D = 128
N_LAYERS = 3
NC = 8
N_OWN = 12544                 # padded rows per core (= 98*128)
N_PAD = N_OWN * NC            # 100352
NQ = 4                        # node-slice quarters (AG chunks / gather tables)
QROWS = N_OWN // NQ           # 3136 rows per rank per quarter
TABROWS = QROWS * NC          # 25088 rows per gather table chunk (< int16 max)
CHUNKS = N_OWN // 128         # 98 matmul chunks
SLOTS = 64                    # dst slots per edge-tile
GROUP = 2                     # tiles per PSUM group (2 x 64 slots = 128)
CALL = 1024                   # rows per dma_gather/scatter call
DUMMY = N_OWN                 # dummy scatter row
XROWS = N_OWN + 128           # accumulator rows (incl. dummy row, 128-aligned)


def _wrap_idx(vals):
    """int16 stream -> [128, n/16] tile layout (16-partition wrap, replicated
    for the 8 Q7 cores). vals length must be a multiple of 16."""
    a = np.asarray(vals, np.int16).reshape(-1, 16).T  # [16, n/16]
    return np.tile(a, (8, 1)).copy()


def _prep_graph(edge_index):
    """Partition/sort/pack edges. Returns per-core input arrays."""
    src = np.asarray(edge_index[0], np.int64)
    dst = np.asarray(edge_index[1], np.int64)
    deg = np.bincount(dst, minlength=N_NODES).astype(np.float32) + 1.0
    dinv = (1.0 / np.sqrt(deg)).astype(np.float32)

    # (self-loop term is folded into the reload phase on-device)
    s_all, d_all = src, dst

    core = d_all // N_OWN
    # two chunked AllGathers (rows [k*6272,(k+1)*6272) of each rank); each
    # 50176-row rank-major chunk output is split into two contiguous
    # 25088-row tables (ranks 0-3 / 4-7). quarter q = 2*chunk + rankhalf.
    s_rank = s_all // N_OWN
    s_i = s_all % N_OWN
    quarter = 2 * (s_i // (2 * QROWS)) + (s_rank // 4)
    tab_row = (s_rank % 4) * (2 * QROWS) + (s_i % (2 * QROWS))
    dst_loc = d_all % N_OWN

    # sort by (core, quarter, dst) so per-(core,quarter) runs are dst-grouped
    order = np.lexsort((d_all, quarter, core))
    core, quarter, tab_row, dst_loc = (
        core[order], quarter[order], tab_row[order], dst_loc[order])

    # pass 1: tile counts per (core, quarter)
    per_cq_tiles = np.zeros((NC, NQ), np.int64)
    cq_edges = {}
    for c in range(NC):
        mc = core == c
        for q in range(NQ):
            m = mc & (quarter == q)
            tr, dl = tab_row[m], dst_loc[m]
            # fragment boundaries (dst changes)
            if dl.size:
                bnd = np.nonzero(np.diff(dl))[0] + 1
                starts = np.concatenate([[0], bnd])
                ends = np.concatenate([bnd, [dl.size]])
            else:
                starts = ends = np.zeros(0, np.int64)
            tiles = []  # each: list of (start, end, dst)
            cur, ce, cs = [], 0, 0
            for s0, e0 in zip(starts, ends):
                fl = e0 - s0
                assert fl <= 128, "dst in-degree fragment exceeds one tile"
                if cur and (ce + fl > 128 or cs + 1 > SLOTS):
                    tiles.append(cur)
                    cur, ce, cs = [], 0, 0
                cur.append((int(s0), int(e0), int(dl[s0])))
                ce += fl
                cs += 1
            if cur:
                tiles.append(cur)
            per_cq_tiles[c, q] = len(tiles)
            cq_edges[(c, q)] = (tr, tiles)

    t_q = int(per_cq_tiles.max())
    # round tiles-per-quarter to a multiple of 16 (one scatter call covers
    # 16 tiles' 64 slots; one gather call covers 8 tiles' edges)
    t_q = (t_q + 15) // 16 * 16
    calls_pq = t_q * 128 // CALL

    zrow = t_q * SLOTS  # zero row in each slot buffer
    gidx = np.zeros((NC, NQ, t_q * 128), np.int16)
    ridx = np.full((NC, NQ, N_OWN), zrow, np.int16)
    ind = np.zeros((NC, NQ, t_q, 128, SLOTS), np.float16)
    for c in range(NC):
        for q in range(NQ):
            tr, tiles = cq_edges[(c, q)]
            for t, frags in enumerate(tiles):
                e = 0
                for j, (s0, e0, d_) in enumerate(frags):
                    fl = e0 - s0
                    gidx[c, q, t * 128 + e : t * 128 + e + fl] = tr[s0:e0]
                    ind[c, q, t, e : e + fl, j] = 1.0
                    tl = t % 16
                    ridx[c, q, d_] = (
                        (t // 16) * 1024
                        + (64 * (tl % 2) + j) * 8
                        + (tl // 8) * 4 + (tl % 8) // 2)
                    e += fl
                # remaining gidx rows stay 0 (valid row, indicator 0)
    return dinv, deg, gidx, ridx, ind, t_q, calls_pq


def _build(nc_prog, t_q, calls_pq, has_bias):
    """Emit the bass program. Returns nothing (tensors declared inside)."""
    nc = nc_prog
    t_tot = t_q * NQ
    # ---------------- I/O ----------------
    xT_in = nc.dram_tensor("xT_in", [128, N_OWN], f16, kind="ExternalInput")
    w_in = [nc.dram_tensor(f"w{l}", [128, 128], f16, kind="ExternalInput")
            for l in range(N_LAYERS)]
    dinv_in = nc.dram_tensor("dinv_in", [128, CHUNKS], f32, kind="ExternalInput")
    gidx_in = nc.dram_tensor("gidx_in", [128, t_tot * 8], i16, kind="ExternalInput")
    ridx_in = nc.dram_tensor("ridx_in", [128, NQ * N_OWN // 16], i16, kind="ExternalInput")
    ind_in = nc.dram_tensor("ind_in", [128, t_tot * SLOTS], f16, kind="ExternalInput")
    ident_in = nc.dram_tensor("ident_in", [128, 128], f16, kind="ExternalInput")
    # gate MLP params
    wg1_in = nc.dram_tensor("wg1_in", [128, 1], f32, kind="ExternalInput")
    bg1_in = nc.dram_tensor("bg1_in", [128, 1], f32, kind="ExternalInput")
    wg2_in = nc.dram_tensor("wg2_in", [128, 128], f32, kind="ExternalInput")
    bg2_in = nc.dram_tensor("bg2_in", [128, 1], f32, kind="ExternalInput")
    ts_in = nc.dram_tensor("ts_in", [128, 1], f32, kind="ExternalInput")
    brep_in = None
    if has_bias:
        brep_in = [nc.dram_tensor(f"brep{l}", [128, 128], f32, kind="ExternalInput")
                   for l in range(N_LAYERS)]
    out_ext = nc.dram_tensor("out_ext", [N_OWN, 128], f32, kind="ExternalOutput")

    # ---------------- internal DRAM ----------------
    h_half = [nc.dram_tensor(f"h_half{k}", [N_OWN // 2, 128], f16)
              for k in range(2)]
    slotbuf = [nc.dram_tensor(f"slotbuf{q}", [t_q * SLOTS + 128, 128], f16)
               for q in range(NQ)]
    h_chunk = [nc.dram_tensor(f"h_chunk{k}", [2 * TABROWS, 128], f16,
                              addr_space="Shared") for k in range(2)]

    with tile.TileContext(nc) as tc:
        with (
            tc.tile_pool(name="const", bufs=1) as cp,
            tc.tile_pool(name="msgp", bufs=12) as msgp,
            tc.tile_pool(name="stgp", bufs=8) as stgp,
            tc.tile_pool(name="hp", bufs=4) as hp,
            tc.tile_pool(name="rp", bufs=4) as rp,
            tc.tile_pool(name="psum_h", bufs=2, space="PSUM") as pp_h,
            tc.tile_pool(name="psum_seg", bufs=4, space="PSUM") as pp_seg,
            tc.tile_pool(name="psum_t", bufs=2, space="PSUM") as pp_t,
        ):
            nc.gpsimd.load_library(library_config.mlp)
            call_reg = nc.gpsimd.to_reg(CALL)
            tail_reg = nc.gpsimd.to_reg(N_OWN % CALL)

            # persistent SBUF
            xT = cp.tile([128, N_OWN], f16)
            nc.sync.dma_start(out=xT[:], in_=xT_in[:])
            wt = []
            for l in range(N_LAYERS):
                w = cp.tile([128, 128], f16, tag=f"w{l}")
                nc.sync.dma_start(out=w[:], in_=w_in[l][:])
                wt.append(w)
            dinv_sb = cp.tile([128, CHUNKS], f32)
            nc.sync.dma_start(out=dinv_sb[:], in_=dinv_in[:])
            gidx = cp.tile([128, t_tot * 8], i16)
            nc.sync.dma_start(out=gidx[:], in_=gidx_in[:])
            ridx = cp.tile([128, NQ * N_OWN // 16], i16)
            nc.sync.dma_start(out=ridx[:], in_=ridx_in[:])
            ident = cp.tile([128, 128], f16)
            nc.sync.dma_start(out=ident[:], in_=ident_in[:])
            brep = []
            if has_bias:
                for l in range(N_LAYERS):
                    bt = cp.tile([128, 128], f32, tag=f"brep{l}")
                    nc.sync.dma_start(out=bt[:], in_=brep_in[l][:])
                    brep.append(bt)
            zeros8 = cp.tile([128, 8, 128], f16)
            nc.vector.memset(zeros8[:], 0.0)
            # zero the slot buffers' zero-row block once
            for q in range(NQ):
                nc.sync.dma_start(
                    out=slotbuf[q][t_q * SLOTS:t_q * SLOTS + 128, :].rearrange(
                        "(c p) f -> p c f", p=128),
                    in_=zeros8[:, :1, :])

            # ---------------- temporal gate ----------------
            wg1 = cp.tile([128, 1], f32)
            nc.sync.dma_start(out=wg1[:], in_=wg1_in[:])
            bg1 = cp.tile([128, 1], f32)
            nc.sync.dma_start(out=bg1[:], in_=bg1_in[:])
            wg2 = cp.tile([128, 128], f32)
            nc.sync.dma_start(out=wg2[:], in_=wg2_in[:])
            bg2 = cp.tile([128, 1], f32)
            nc.sync.dma_start(out=bg2[:], in_=bg2_in[:])
            tsr = cp.tile([128, 1], f32)
            nc.sync.dma_start(out=tsr[:], in_=ts_in[:])

            tmp1 = cp.tile([128, 1], f32, tag="g1")
            nc.vector.tensor_mul(tmp1[:], wg1[:], tsr[:])
            tanh1 = cp.tile([128, 1], f32, tag="g2")
            nc.scalar.activation(
                tanh1[:], tmp1[:], mybir.ActivationFunctionType.Tanh, bias=bg1[:])
            ps_g = pp_h.tile([128, 1], f32, tag="psh")
            nc.tensor.matmul(ps_g[:], lhsT=wg2[:], rhs=tanh1[:], start=True, stop=True)
            gate_col = cp.tile([128, 1], f32, tag="gcol")
            nc.scalar.activation(
                gate_col[:], ps_g[:], mybir.ActivationFunctionType.Sigmoid,
                bias=bg2[:])
            gate_col16 = cp.tile([128, 1], f16, tag="gcol16")
            nc.scalar.activation(
                gate_col16[:], ps_g[:], mybir.ActivationFunctionType.Sigmoid,
                bias=bg2[:])
            # replicate gate over partitions: transpose to row, K=1 matmul
            ps_gr = pp_t.tile([1, 128], f16, tag="pst")
            nc.tensor.transpose(out=ps_gr[:], in_=gate_col16[:], identity=ident[:])
            gate_row = cp.tile([1, 128], f16, tag="grow")
            nc.vector.tensor_copy(out=gate_row[:], in_=ps_gr[:])
            ones_row = cp.tile([1, 128], f16, tag="ones")
            nc.vector.memset(ones_row[:], 1.0)
            ps_rep = pp_seg.tile([128, 128], f32, tag="pseg")
            nc.tensor.matmul(
                ps_rep[:], lhsT=ones_row[:], rhs=gate_row[:], start=True, stop=True)
            gate_rep = cp.tile([128, 128], f16, tag="grep")
            nc.scalar.activation(
                gate_rep[:], ps_rep[:], mybir.ActivationFunctionType.Copy)

            # ---------------- layers ----------------
            for l in range(N_LAYERS):
                # h' = dinv * (x @ W)  -> h_bounce
                for c4 in range(CHUNKS // 4 + (1 if CHUNKS % 4 else 0)):
                    n4 = min(4, CHUNKS - c4 * 4)
                    h4 = hp.tile([128, 4, 128], f16, tag="h4")
                    for j in range(n4):
                        c = c4 * 4 + j
                        ps_h = pp_h.tile([128, 128], f32, tag="psh")
                        nc.tensor.matmul(
                            ps_h[:], lhsT=xT[:, c * 128:(c + 1) * 128],
                            rhs=wt[l][:], start=True, stop=True)
                        nc.scalar.activation(
                            h4[:, j, :], ps_h[:],
                            mybir.ActivationFunctionType.Copy,
                            scale=dinv_sb[:, c:c + 1])
                    # write to the owning half (chunks 0-48 -> half 0)
                    c0 = c4 * 4
                    spans = []
                    if c0 < 49:
                        spans.append((0, c0, min(c0 + n4, 49)))
                    if c0 + n4 > 49:
                        spans.append((1, max(c0, 49), c0 + n4))
                    for (hk, ca, cb) in spans:
                        base = hk * 49
                        nc.sync.dma_start(
                            out=h_half[hk][(ca - base) * 128:(cb - base) * 128, :]
                            .rearrange("(c p) f -> p c f", p=128),
                            in_=h4[:, ca - c0:cb - c0, :])

                # two chunked AllGathers; gathers for a chunk's two tables
                # start as soon as that chunk lands
                for k in range(2):
                    nc.gpsimd.collective_compute(
                        "AllGather", mybir.AluOpType.bypass,
                        replica_groups=[list(range(NC))],
                        ins=[h_half[k][:]],
                        outs=[h_chunk[k][:]],
                    )


                # gather -> segment matmul -> dense slot-row eviction, per quarter.
                # One scatter call covers 16 tiles (64 slots each) = 8 PSUM
                # groups = 2 gather calls. stg slice j holds PSUM group j's
                # 128 slot rows (scatter stream position i -> [i%128, i//128]).
                for q in range(NQ):
                    for sc in range(t_q // 16):
                        stg = stgp.tile([128, 8, 128], f16, tag="stg")
                        unit0 = q * t_q + sc * 16
                        indb = msgp.tile([128, 16, SLOTS], f16, tag="indb")
                        nc.sync.dma_start(
                            out=indb[:],
                            in_=ind_in[:, unit0 * SLOTS:(unit0 + 16) * SLOTS]
                            .rearrange("p (t s) -> p t s", s=SLOTS))
                        for gc in range(2):
                            call = sc * 2 + gc
                            tile0 = unit0 + gc * 8
                            msg = msgp.tile([128, 8, 128], f16, tag="msg")
                            gcol0 = tile0 * 8  # int16 cols (128 idx = 8 cols)
                            nc.gpsimd.dma_gather(
                                out_ap=msg[:],
                                in_ap=h_chunk[q // 2][
                                    (q % 2) * TABROWS:(q % 2 + 1) * TABROWS, :],
                                idxs_ap=gidx[:, gcol0:gcol0 + 64],
                                num_idxs=CALL, num_idxs_reg=call_reg, elem_size=128,
                                queue_num=call % NQ)
                            for g in range(4):  # 4 psum groups of 2 tiles
                                ps = pp_seg.tile([128, 128], f32, tag="pseg")
                                for j in range(GROUP):
                                    tl = gc * 8 + g * 2 + j
                                    nc.tensor.matmul(
                                        ps[64 * j:64 * (j + 1), :],
                                        lhsT=indb[:, tl, :],
                                        rhs=msg[:, g * 2 + j, :],
                                        start=True, stop=True,
                                        tile_position=(0, 64 * j))
                                nc.vector.tensor_copy(
                                    out=stg[:, gc * 4 + g, :], in_=ps[:])
                        (nc.sync if sc % 2 == 0 else nc.scalar).dma_start(
                            out=slotbuf[q][sc * CALL:(sc + 1) * CALL, :].rearrange(
                                "(p c) f -> p c f", p=128),
                            in_=stg[:])

                # reload: gather each node's slot rows from the 4 quarter
                # slot buffers, merge (+ self term h'), relu*dinv, transpose
                last = l == N_LAYERS - 1
                for u in range((N_OWN + CALL - 1) // CALL):
                    r0 = u * CALL
                    nrow = min(CALL, N_OWN - r0)
                    n8 = nrow // 128
                    nreg = call_reg if nrow == CALL else tail_reg
                    th = rp.tile([128, 8, 128], f16, tag="ldh")
                    hb = N_OWN // 2
                    spans_r = []
                    if r0 < hb:
                        spans_r.append((0, r0, min(r0 + nrow, hb)))
                    if r0 + nrow > hb:
                        spans_r.append((1, max(r0, hb), r0 + nrow))
                    for (hk, ra, rb) in spans_r:
                        nc.scalar.dma_start(
                            out=th[:, (ra - r0) // 128:(rb - r0) // 128, :],
                            in_=h_half[hk][ra - hk * hb:rb - hk * hb, :]
                            .rearrange("(c p) f -> p c f", p=128))
                    acc = rp.tile([128, 8, 128], f16, tag="acc")
                    for q in range(NQ):
                        rg = rp.tile([128, 8, 128], f16, tag=f"rg{q}")
                        rcol0 = (q * N_OWN + r0) // 16
                        nc.gpsimd.dma_gather(
                            out_ap=rg[:, :n8, :],
                            in_ap=slotbuf[q][:],
                            idxs_ap=ridx[:, rcol0:rcol0 + nrow // 16],
                            num_idxs=nrow, num_idxs_reg=nreg, elem_size=128,
                            queue_num=(u + q) % NQ)
                        if q == 0:
                            nc.vector.tensor_add(
                                acc[:, :n8, :], th[:, :n8, :], rg[:, :n8, :])
                        else:
                            nc.vector.tensor_add(
                                acc[:, :n8, :], acc[:, :n8, :], rg[:, :n8, :])
                    for j in range(n8):
                        c = (r0 // 128) + j
                        if has_bias:
                            pre = rp.tile([128, 128], f32, tag="pre")
                            nc.vector.tensor_scalar_mul(
                                pre[:], acc[:, j, :], dinv_sb[:, c:c + 1])
                            nc.vector.tensor_add(pre[:], pre[:], brep[l][:])
                            src_ap, src_scale = pre, None
                        else:
                            src_ap, src_scale = None, dinv_sb[:, c:c + 1]
                        if last:
                            row32 = rp.tile([128, 128], f32, tag="row32")
                            nc.scalar.activation(
                                row32[:], pre[:] if has_bias else acc[:, j, :],
                                mybir.ActivationFunctionType.Relu,
                                scale=1.0 if has_bias else dinv_sb[:, c:c + 1])
                            rowo = rp.tile([128, 128], f32, tag="rowo")
                            nc.vector.tensor_mul(
                                rowo[:], row32[:], gate_rep[:])
                            nc.sync.dma_start(
                                out=out_ext[c * 128:(c + 1) * 128, :],
                                in_=rowo[:])
                        else:
                            xr2 = rp.tile([128, 128], f16, tag="xr2")
                            nc.scalar.activation(
                                xr2[:], pre[:] if has_bias else acc[:, j, :],
                                mybir.ActivationFunctionType.Relu,
                                scale=1.0 if has_bias else dinv_sb[:, c:c + 1])
                            ps_t = pp_t.tile([128, 128], f16, tag="pst")
                            nc.tensor.transpose(
                                out=ps_t[:], in_=xr2[:], identity=ident[:])
                            nc.vector.tensor_scalar_mul(
                                xT[:, c * 128:(c + 1) * 128], ps_t[:],
                                gate_col[:])

    mybir.codegen_inst_isa_subclasses(nc)


_CACHE = {}


def _get_program(t_q, calls_pq, has_bias):
    key = (t_q, calls_pq, has_bias)
    if key not in _CACHE:
        nc = bass.Bass(num_devices=NC, num_swdge_queues=NQ)
        _build(nc, t_q, calls_pq, has_bias)
        _CACHE[key] = nc
    return _CACHE[key]


def _prepare(inputs):
    x = np.asarray(inputs["x"], np.float32)
    edge_index = np.asarray(inputs["edge_index"])
    ts = np.asarray(inputs["timestamp"], np.float32).reshape(-1)[0]
    Ws = [np.asarray(inputs[f"W{l}"], np.float32) for l in range(N_LAYERS)]
    bs = [np.asarray(inputs[f"b{l}"], np.float32) for l in range(N_LAYERS)]
    Wg1 = np.asarray(inputs["Wg1"], np.float32)
    bg1 = np.asarray(inputs["bg1"], np.float32)
    Wg2 = np.asarray(inputs["Wg2"], np.float32)
    bg2 = np.asarray(inputs["bg2"], np.float32)

    dinv, deg, gidx, ridx, ind, t_q, calls_pq = _prep_graph(edge_index)
    has_bias = any(np.abs(b).max() > 0 for b in bs)

    ident = np.eye(128, dtype=np.float16)
    in_maps = []
    for c in range(NC):
        lo = c * N_OWN
        hi = min((c + 1) * N_OWN, N_NODES)
        xb = np.zeros((N_OWN, 128), np.float16)
        xb[: hi - lo] = x[lo:hi].astype(np.float16)
        dv = np.ones(N_OWN, np.float32)
        dv[: hi - lo] = dinv[lo:hi]
        m = {
            "xT_in": np.ascontiguousarray(xb.T),
            "dinv_in": np.ascontiguousarray(dv.reshape(CHUNKS, 128).T),
            "gidx_in": np.concatenate(
                [_wrap_idx(gidx[c, q]) for q in range(NQ)], axis=1),
            "ridx_in": np.concatenate(
                [_wrap_idx(ridx[c, q]) for q in range(NQ)], axis=1),
            "ind_in": np.ascontiguousarray(
                ind[c].reshape(NQ * t_q, 128, SLOTS).transpose(1, 0, 2)
                .reshape(128, NQ * t_q * SLOTS)),
            "ident_in": ident,
            "wg1_in": Wg1.reshape(128, 1),
            "bg1_in": bg1.reshape(128, 1),
            "wg2_in": np.ascontiguousarray(Wg2),
            "bg2_in": bg2.reshape(128, 1),
            "ts_in": np.full((128, 1), ts, np.float32),
        }
        for l in range(N_LAYERS):
            m[f"w{l}"] = Ws[l].astype(np.float16)
            if has_bias:
                m[f"brep{l}"] = np.tile(bs[l].astype(np.float32), (128, 1))
        in_maps.append(m)
    return in_maps, t_q, calls_pq, has_bias


def _run(inputs, trace=False):
    in_maps, t_q, calls_pq, has_bias = _prepare(inputs)
    nc = _get_program(t_q, calls_pq, has_bias)
    res = run_bass_kernel_spmd(
        nc, in_maps, core_ids=list(range(NC)), trace=trace)
    blocks = [res.results[c]["out_ext"] for c in range(NC)]
    out = np.concatenate(blocks, axis=0)[:N_NODES].astype(np.float32)
    return out, res


def kernel(**inputs) -> np.ndarray:
    out, _ = _run(inputs, trace=False)
    return out


def kernel_traced(**inputs):
    return _run(inputs, trace=True)
